# revision 10
# baseline (speedup 1.0000x reference)
"""Additive (Bahdanau) attention on 8 TRN2 NeuronCores (raw Bass).

Reference math (B=4, Tq=256, Tk=512, Dq=Dv=512, U=256):
    q = query @ W1                      [B,Tq,U]
    k = value @ W2                      [B,Tk,U]
    scores[b,t,s] = sum_u scale[u] * tanh(q[b,t,u] + k[b,s,u])
    attn = softmax(scores, axis=-1)     [B,Tq,Tk]
    context = attn @ value              [B,Tq,Dv]
    returns (context, attn)

Sharding: (b, tq-half) -> 8 cores, 128 query rows each; Tk stays local so
there are no collectives.  Per-core dataflow keeps U on partitions for the
big [t,s,u] stage:
    DVE:  X[u, (t,s)] = k[u,s] + q[u,t]   (tensor_scalar add, bf16 4x mode)
    ACT:  T = tanh(X)                     (one big activation per t-block)
    PE :  scoresT[s,t] = sum_u scale[u] T[u,s]   (per-t matvecs, T stationary)
    ACT:  E = exp(scoresT)                (softmax without max: |scores|<~13)
    PE :  sums[t] = E.T @ 1, ctx_raw = E.T @ value, attnT = transpose(E)
    DVE:  r = 1/sums; outputs scaled by r (per-partition scalar)

Raw bass (explicit engine programs + semaphores): the walrus build in this
environment only allows one attached sync-wait per instruction, so waits are
standalone wait_ge instructions per engine.
"""

from contextlib import ExitStack

import numpy as np

import concourse.bass as bass
import concourse.mybir as mybir
from concourse.bass_utils import run_bass_kernel_spmd

F32 = mybir.dt.float32
BF16 = mybir.dt.bfloat16
AF = mybir.ActivationFunctionType

N_CORES = 8
B, TQ, TK, DQ, DV, U = 4, 256, 512, 512, 512, 256
T_ROWS = 128          # query rows per core
UC = U // 128         # u chunks (2)
DC = DQ // 128        # d chunks (4)
SC = TK // 128        # s chunks (4)
TB = 8                # t-block size for the tanh pipeline
NTB = T_ROWS // TB    # 16
XFREE = UC * TB * TK  # 8192 free elems per X/T buffer


def build_bass() -> bass.Bass:
    nc = bass.Bass()
    q_ext = nc.declare_dram_parameter("query", [T_ROWS, DQ], F32, isOutput=False)
    w1_ext = nc.declare_dram_parameter("W1", [DQ, U], F32, isOutput=False)
    w2_ext = nc.declare_dram_parameter("W2", [DV, U], F32, isOutput=False)
    v_ext = nc.declare_dram_parameter("value", [TK, DV], F32, isOutput=False)
    scale_ext = nc.declare_dram_parameter("scale", [128, UC], F32, isOutput=False)
    ctx_ext = nc.declare_dram_parameter("context", [T_ROWS, DV], F32, isOutput=True)
    attn_ext = nc.declare_dram_parameter("attn", [T_ROWS, TK], F32, isOutput=True)

    es = ExitStack()
    with es:
        _sbn = [0]
        def sb(shape, dt, name=None):
            _sbn[0] += 1
            return es.enter_context(nc.sbuf_tensor(name or f"sb{_sbn[0]}", shape, dt))
        # ---- SBUF ----
        q_sb = sb([128, DQ], F32)              # query [t, d]
        v_sb = sb([128, SC * DV], F32)         # value [s_p, (sc, d)]
        w1_sb = sb([128, DC * U], F32)         # [d_p, (dc, u)]
        w2_sb = sb([128, DC * U], F32)
        scale_sb = sb([128, UC], F32)
        scale_bf = sb([128, UC], BF16)
        v_bf = sb([128, SC * DV], BF16)
        ones_bf = sb([128, 1], BF16)
        ident_f = sb([128, 128], F32)
        ident_bf = sb([128, 128], BF16)
        qT_sb = sb([128, DC * 128], F32)       # [d_p, (dc, t)]
        vT_sb = sb([128, DC * TK], F32)        # [d_p, (dc, s)]
        q_f = sb([128, UC * 128], F32)         # [u_p, (uc, t)]
        k_bf = sb([128, UC * TK], BF16)        # [u_p, (uc, s)]
        X0 = sb([128, XFREE], BF16)
        X1 = sb([128, XFREE], BF16)
        T0 = sb([128, XFREE], BF16)
        T1 = sb([128, XFREE], BF16)
        E_bf = sb([128, SC * 128], BF16)       # [s_p, (sc, t)]
        r_sb = sb([128, 1], F32)
        ctx_sb = sb([128, DV], F32)
        attn_sb = sb([128, TK], F32)
        Xs, Ts = [X0, X1], [T0, T1]

        # ---- PSUM: 8 banks of [128, 512] f32 ----
        banks = [
            es.enter_context(nc.psum_tensor(f"bank{i}", [128, 512], F32))
            for i in range(8)
        ]
        scores_ps = [banks[i][:, 0:T_ROWS] for i in range(4)]   # [s_p, t]
        trans_ps = [banks[4][:, 0:128], banks[5][:, 0:128]]     # transpose ping/pong
        q_ps = [banks[4][:, 0:128], banks[5][:, 0:128]]
        k_ps = [banks[6][:, 0:TK], banks[7][:, 0:TK]]
        sums_ps = banks[4][:, 0:1]
        ctx_ps = banks[6][:, 0:DV]
        # attnT (bf16 views): [b7.lo, b5.lo, b7.hi, b5.hi]
        attnT_ps = [
            banks[7][:, 0:64].bitcast(BF16),
            banks[5][:, 0:64].bitcast(BF16),
            banks[7][:, 64:128].bitcast(BF16),
            banks[5][:, 64:128].bitcast(BF16),
        ]

        sem = lambda name: es.enter_context(nc.semaphore(name))
        s_dma = sem("s_dma")
        s_ident = sem("s_ident")
        s_cast = sem("s_cast")
        s_trans = sem("s_trans")
        s_proj = sem("s_proj")
        s_evac = sem("s_evac")
        s_add = sem("s_add")
        s_tanh = sem("s_tanh")
        s_mv = sem("s_mv")
        s_exp = sem("s_exp")
        s_sums = sem("s_sums")
        s_ctx = sem("s_ctx")
        s_att = sem("s_att")
        s_out = sem("s_out")
        s_dout = sem("s_dout")

        with nc.Block() as block:

            @block.sync
            def _(sync):
                sync.dma_start(out=q_sb[:, :], in_=q_ext[:, :]).then_inc(s_dma, 16)
                sync.dma_start(
                    out=w1_sb[:, :].rearrange("p (dc u) -> p dc u", dc=DC),
                    in_=w1_ext[:, :].rearrange("(dc p) u -> p dc u", p=128),
                ).then_inc(s_dma, 16)
                sync.dma_start(
                    out=w2_sb[:, :].rearrange("p (dc u) -> p dc u", dc=DC),
                    in_=w2_ext[:, :].rearrange("(dc p) u -> p dc u", p=128),
                ).then_inc(s_dma, 16)
                sync.dma_start(
                    out=v_sb[:, :].rearrange("p (sc d) -> p sc d", sc=SC),
                    in_=v_ext[:, :].rearrange("(sc p) d -> p sc d", p=128),
                ).then_inc(s_dma, 16)
                sync.dma_start(out=scale_sb[:, :], in_=scale_ext[:, :]).then_inc(
                    s_dma, 16
                )
                sync.wait_ge(s_out, 1)
                sync.dma_start(out=ctx_ext[:, :], in_=ctx_sb[:, :]).then_inc(
                    s_dout, 16
                )
                sync.wait_ge(s_out, 5)
                sync.dma_start(out=attn_ext[:, :], in_=attn_sb[:, :]).then_inc(
                    s_dout, 16
                )
                sync.wait_ge(s_dout, 32)

            @block.gpsimd
            def _(gpsimd):
                gpsimd.memset(ident_f[:, :], 0.0)
                gpsimd.affine_select(
                    out=ident_f[:, :],
                    in_=ident_f[:, :],
                    compare_op=mybir.AluOpType.not_equal,
                    fill=1.0,
                    base=0,
                    pattern=[[-1, 128]],
                    channel_multiplier=1,
                ).then_inc(s_ident, 1)
                gpsimd.memset(ident_bf[:, :], 0.0)
                gpsimd.affine_select(
                    out=ident_bf[:, :],
                    in_=ident_bf[:, :],
                    compare_op=mybir.AluOpType.not_equal,
                    fill=1.0,
                    base=0,
                    pattern=[[-1, 128]],
                    channel_multiplier=1,
                ).then_inc(s_ident, 1)

            @block.vector
            def _(vector):
                vector.wait_ge(s_dma, 80)
                vector.tensor_copy(out=v_bf[:, :], in_=v_sb[:, :]).then_inc(s_cast, 1)
                vector.tensor_copy(out=scale_bf[:, :], in_=scale_sb[:, :]).then_inc(
                    s_cast, 1
                )
                vector.memset(ones_bf[:, :], 1.0).then_inc(s_cast, 1)
                # phase 1: broadcast adds
                vector.wait_ge(s_evac, 24)
                for tb in range(NTB):
                    buf = Xs[tb % 2]
                    if tb >= 2:
                        vector.wait_ge(s_tanh, tb - 1)
                    for tl in range(TB):
                        t = tb * TB + tl
                        for uc in range(UC):
                            ins = vector.tensor_scalar_add(
                                out=buf[:, (tl * UC + uc) * TK : (tl * UC + uc + 1) * TK],
                                in0=k_bf[:, uc * TK : (uc + 1) * TK],
                                scalar1=q_f[:, uc * 128 + t : uc * 128 + t + 1],
                            )
                            if tl == TB - 1 and uc == UC - 1:
                                ins.then_inc(s_add, 1)
                # phase 2 epilogue
                vector.wait_ge(s_sums, 1)
                vector.reciprocal(out=r_sb[:, :], in_=sums_ps)
                vector.wait_ge(s_ctx, 1)
                vector.tensor_scalar_mul(
                    out=ctx_sb[:, :], in0=ctx_ps, scalar1=r_sb[:, 0:1]
                ).then_inc(s_out, 1)
                for i in range(4):
                    vector.wait_ge(s_att, i + 1)
                    vector.tensor_scalar_mul(
                        out=attn_sb[:, i * 128 : (i + 1) * 128],
                        in0=attnT_ps[i],
                        scalar1=r_sb[:, 0:1],
                    ).then_inc(s_out, 1)

            @block.scalar
            def _(scalar):
                # phase 0 psum evacuations (funnelled through ACT)
                for i in range(DC):  # q transposes -> qT
                    scalar.wait_ge(s_trans, i + 1)
                    scalar.copy(
                        out=qT_sb[:, i * 128 : (i + 1) * 128], in_=trans_ps[i % 2]
                    ).then_inc(s_evac, 1)
                for j in range(SC * DC):  # v transposes -> vT
                    i = DC + j
                    sc, dc = j // DC, j % DC
                    scalar.wait_ge(s_trans, i + 1)
                    scalar.copy(
                        out=vT_sb[:, dc * TK + sc * 128 : dc * TK + sc * 128 + 128],
                        in_=trans_ps[i % 2],
                    ).then_inc(s_evac, 1)
                for uc in range(UC):
                    scalar.wait_ge(s_proj, uc + 1)
                    scalar.copy(
                        out=q_f[:, uc * 128 : (uc + 1) * 128], in_=q_ps[uc]
                    ).then_inc(s_evac, 1)
                for uc in range(UC):
                    scalar.wait_ge(s_proj, 3 + uc)
                    scalar.copy(
                        out=k_bf[:, uc * TK : (uc + 1) * TK], in_=k_ps[uc]
                    ).then_inc(s_evac, 1)
                # phase 1: tanh
                for tb in range(NTB):
                    scalar.wait_ge(s_add, tb + 1)
                    if tb >= 2:
                        scalar.wait_ge(s_mv, tb - 1)
                    scalar.activation(
                        out=Ts[tb % 2][:, :], in_=Xs[tb % 2][:, :], func=AF.Tanh
                    ).then_inc(s_tanh, 1)
                # phase 2: exp
                scalar.wait_ge(s_mv, NTB)
                for sc in range(SC):
                    scalar.activation(
                        out=E_bf[:, sc * 128 : (sc + 1) * 128],
                        in_=scores_ps[sc],
                        func=AF.Exp,
                    ).then_inc(s_exp, 1)

            @block.tensor
            def _(tensor):
                tensor.wait_ge(s_dma, 80)
                tensor.wait_ge(s_ident, 2)
                tensor.wait_ge(s_cast, 3)
                # transposes: 4 of query, 16 of value, ping-pong banks 4/5
                for i in range(DC):
                    if i >= 2:
                        tensor.wait_ge(s_evac, i - 1)
                    tensor.transpose(
                        out=trans_ps[i % 2],
                        in_=q_sb[:, i * 128 : (i + 1) * 128],
                        identity=ident_f[:, :],
                    ).then_inc(s_trans, 1)
                for j in range(SC * DC):
                    i = DC + j
                    sc, dc = j // DC, j % DC
                    if i >= 2:
                        tensor.wait_ge(s_evac, i - 1)
                    tensor.transpose(
                        out=trans_ps[i % 2],
                        in_=v_sb[:, sc * DV + dc * 128 : sc * DV + dc * 128 + 128],
                        identity=ident_f[:, :],
                    ).then_inc(s_trans, 1)
                # projections (fp32)
                for uc in range(UC):  # qT[u,t] into banks 4/5
                    tensor.wait_ge(s_evac, 19 + uc)
                    for dc in range(DC):
                        ins = tensor.matmul(
                            out=q_ps[uc],
                            lhsT=w1_sb[:, dc * U + uc * 128 : dc * U + uc * 128 + 128],
                            rhs=qT_sb[:, dc * 128 : (dc + 1) * 128],
                            start=(dc == 0),
                            stop=(dc == DC - 1),
                        )
                    ins.then_inc(s_proj, 1)
                for uc in range(UC):  # kT[u,s] into banks 6/7
                    for dc in range(DC):
                        ins = tensor.matmul(
                            out=k_ps[uc],
                            lhsT=w2_sb[:, dc * U + uc * 128 : dc * U + uc * 128 + 128],
                            rhs=vT_sb[:, dc * TK : (dc + 1) * TK],
                            start=(dc == 0),
                            stop=(dc == DC - 1),
                        )
                    ins.then_inc(s_proj, 1)
                # phase 1: score matvecs
                for tb in range(NTB):
                    tensor.wait_ge(s_tanh, tb + 1)
                    Tt = Ts[tb % 2]
                    for tl in range(TB):
                        t = tb * TB + tl
                        for sc in range(SC):
                            for uc in range(UC):
                                base = (tl * UC + uc) * TK + sc * 128
                                ins = tensor.matmul(
                                    out=scores_ps[sc][:, t : t + 1],
                                    lhsT=Tt[:, base : base + 128],
                                    rhs=scale_bf[:, uc : uc + 1],
                                    start=(uc == 0),
                                    stop=(uc == UC - 1),
                                )
                    ins.then_inc(s_mv, 1)
                # phase 2
                tensor.wait_ge(s_exp, 4)
                for sc in range(SC):  # sums[t] into bank 4
                    ins = tensor.matmul(
                        out=sums_ps,
                        lhsT=E_bf[:, sc * 128 : (sc + 1) * 128],
                        rhs=ones_bf[:, 0:1],
                        start=(sc == 0),
                        stop=(sc == SC - 1),
                    )
                ins.then_inc(s_sums, 1)
                for sc in range(SC):  # context into bank 6
                    ins = tensor.matmul(
                        out=ctx_ps,
                        lhsT=E_bf[:, sc * 128 : (sc + 1) * 128],
                        rhs=v_bf[:, sc * DV : (sc + 1) * DV],
                        start=(sc == 0),
                        stop=(sc == SC - 1),
                    )
                ins.then_inc(s_ctx, 1)
                for sc in range(4):  # attnT transposes
                    if sc >= 2:
                        # banks 7/5 are reused for sc=2,3: wait until DVE has
                        # consumed the sc-2 tile (avoids PE-W + DVE-R same bank)
                        tensor.wait_ge(s_out, sc)
                    tensor.transpose(
                        out=attnT_ps[sc],
                        in_=E_bf[:, sc * 128 : (sc + 1) * 128],
                        identity=ident_bf[:, :],
                    ).then_inc(s_att, 1)

    return nc


_NC = None


def _get_nc() -> bass.Bass:
    global _NC
    if _NC is None:
        _NC = build_bass()
    return _NC


def make_in_maps(query, value, W1, W2, scale):
    query = np.asarray(query, dtype=np.float32)
    value = np.asarray(value, dtype=np.float32)
    W1 = np.ascontiguousarray(np.asarray(W1, dtype=np.float32))
    W2 = np.ascontiguousarray(np.asarray(W2, dtype=np.float32))
    scale_t = np.ascontiguousarray(
        np.asarray(scale, dtype=np.float32).reshape(UC, 128).T
    )
    in_maps = []
    for c in range(N_CORES):
        b, th = c // 2, c % 2
        in_maps.append(
            {
                "query": np.ascontiguousarray(
                    query[b, th * T_ROWS : (th + 1) * T_ROWS, :]
                ),
                "value": np.ascontiguousarray(value[b]),
                "W1": W1,
                "W2": W2,
                "scale": scale_t,
            }
        )
    return in_maps


def assemble(results):
    context = np.empty((B, TQ, DV), dtype=np.float32)
    attn = np.empty((B, TQ, TK), dtype=np.float32)
    for c in range(N_CORES):
        b, th = c // 2, c % 2
        context[b, th * T_ROWS : (th + 1) * T_ROWS, :] = results[c]["context"]
        attn[b, th * T_ROWS : (th + 1) * T_ROWS, :] = results[c]["attn"]
    return context, attn


def kernel(query, value, W1, W2, scale):
    nc = _get_nc()
    in_maps = make_in_maps(query, value, W1, W2, scale)
    res = run_bass_kernel_spmd(nc, in_maps, core_ids=list(range(N_CORES)))
    return assemble(res.results)


# revision 15
# speedup vs baseline: 1.0023x; 1.0023x over previous
"""Additive (Bahdanau) attention on 8 TRN2 NeuronCores (raw Bass).

Reference math (B=4, Tq=256, Tk=512, Dq=Dv=512, U=256):
    q = query @ W1                      [B,Tq,U]
    k = value @ W2                      [B,Tk,U]
    scores[b,t,s] = sum_u scale[u] * tanh(q[b,t,u] + k[b,s,u])
    attn = softmax(scores, axis=-1)     [B,Tq,Tk]
    context = attn @ value              [B,Tq,Dv]
    returns (context, attn)

Sharding: (b, tq-half) -> 8 cores, 128 query rows each; Tk stays local so
there are no collectives.  Per-core dataflow keeps U on partitions for the
big [t,s,u] stage:
    DVE:  X[u, (t,s)] = k[u,s] + q[u,t]   (tensor_scalar add, bf16 4x mode)
    ACT:  T = tanh(X)                     (one big activation per t-block)
    PE :  scoresT[s,t] = sum_u scale[u] T[u,s]   (per-t matvecs, T stationary)
    ACT:  E = exp(scoresT)                (softmax without max: |scores|<~13)
    PE :  sums[t] = E.T @ 1, ctx_raw = E.T @ value, attnT = transpose(E)
    DVE:  r = 1/sums; outputs scaled by r (per-partition scalar)

Pipeline notes:
  - walrus here allows only ONE attached sync-wait per instruction, so all
    waits are standalone wait_ge instructions per engine (raw bass).
  - per-input-DMA semaphores: HWDGE completions are NOT FIFO across DMAs, so
    cumulative counts are only used when waiting for ALL DMAs on that sem.
  - inputs spread over sync HWDGE (v0,v1,q), scalar HWDGE (idf,v2,v3,idb)
    and gpsimd SWDGE (w2,w1,scale) rings; identities arrive via DMA.
  - PE transposes fill 4-bank PSUM rings (groups sc0,sc1,sc2,sc3,q), each
    evacuated in one strided DVE copy; the k projection starts as soon as
    vT is complete so the tanh pipeline starts ~12us in.
  - scores land in PSUM quarter tiles (t in blocks of 32); softmax + context
    + attn-transpose + output DMA for quarter i run under the tanh stream of
    later blocks, so only the last quarter is in the tail.
"""

from contextlib import ExitStack

import numpy as np

import concourse.bass as bass
import concourse.mybir as mybir
from concourse.bass_utils import run_bass_kernel_spmd

F32 = mybir.dt.float32
BF16 = mybir.dt.bfloat16
AF = mybir.ActivationFunctionType

N_CORES = 8
B, TQ, TK, DQ, DV, U = 4, 256, 512, 512, 512, 256
T_ROWS = 128          # query rows per core
TQR = 32              # t-quarter size
UC = U // 128         # u chunks (2)
DC = DQ // 128        # d chunks (4)
SC = TK // 128        # s chunks (4)
TB = 8                # t-block size for the tanh pipeline
NTB = T_ROWS // TB    # 16
XFREE = UC * TB * TK  # 8192 free elems per X/T buffer


def build_bass() -> bass.Bass:
    nc = bass.Bass()
    q_ext = nc.declare_dram_parameter("query", [T_ROWS, DQ], F32, isOutput=False)
    w1_ext = nc.declare_dram_parameter("W1", [DQ, U], F32, isOutput=False)
    w2_ext = nc.declare_dram_parameter("W2", [DV, U], F32, isOutput=False)
    v_ext = nc.declare_dram_parameter("value", [TK, DV], F32, isOutput=False)
    scale_ext = nc.declare_dram_parameter("scale", [128, UC], F32, isOutput=False)
    idf_ext = nc.declare_dram_parameter("identf", [128, 128], F32, isOutput=False)
    idb_ext = nc.declare_dram_parameter("identb", [128, 128], BF16, isOutput=False)
    ctx_ext = nc.declare_dram_parameter("context", [T_ROWS, DV], F32, isOutput=True)
    attn_ext = nc.declare_dram_parameter("attn", [T_ROWS, TK], F32, isOutput=True)

    es = ExitStack()
    with es:
        _n = [0]

        def sb(shape, dt):
            _n[0] += 1
            return es.enter_context(nc.sbuf_tensor(f"sb{_n[0]}", shape, dt))

        # ---- SBUF ----
        q_sb = sb([128, DQ], F32)              # query [t, d]
        v_sb = sb([128, SC * DV], F32)         # value [s_p, (sc, d)]
        w1_sb = sb([128, DC * U], F32)         # [d_p, (dc, u)]
        w2_sb = sb([128, DC * U], F32)
        scale_sb = sb([128, UC], F32)
        scale_bf = sb([128, UC], BF16)
        v_bf = sb([128, SC * DV], BF16)
        ones_bf = sb([128, 1], BF16)
        ident_f = sb([128, 128], F32)
        ident_bf = sb([128, 128], BF16)
        qT_sb = sb([128, DC * 128], F32)       # [d_p, (dc, t)]
        vT_sb = sb([128, DC * TK], F32)        # [d_p, (dc, s)]
        q_f = sb([128, UC * 128], F32)         # [u_p, (uc, t)]
        k_bf = sb([128, UC * TK], BF16)        # [u_p, (uc, s)]
        X0 = sb([128, XFREE], BF16)
        X1 = sb([128, XFREE], BF16)
        T0 = sb([128, XFREE], BF16)
        T1 = sb([128, XFREE], BF16)
        E_Q = [sb([128, SC * TQR], BF16) for _ in range(4)]  # [s_p, (sc, t32)]
        r_Q = [sb([128, 1], F32) for _ in range(4)]
        ctx_Q = [sb([128, DV], F32) for _ in range(4)]       # rows 0:32 used
        attn_Q = [sb([128, TK], F32) for _ in range(4)]
        Xs, Ts = [X0, X1], [T0, T1]

        # ---- PSUM: two 4-bank rings ----
        ringA = es.enter_context(nc.psum_tensor("ringA", [128, 2048], F32))
        ringB = es.enter_context(nc.psum_tensor("ringB", [128, 2048], F32))
        # transposes: block j of a group -> ring[:, j*512 : j*512+128]
        # scores quarter i -> ringA[:, (i%2)*512 + (i//2)*128 : +128]
        #   (4 sc tiles of 32 cols each)
        k_ps = [ringB[:, 1024:1536], ringB[:, 1536:2048]]
        q_ps = [ringB[:, 0:128], ringB[:, 512:640]]
        sums_Q = [
            ringB[0:TQR, 1024:1025],
            ringB[0:TQR, 0:1],
            ringB[0:TQR, 1024:1025],
            ringB[0:TQR, 0:1],
        ]
        ctxp_Q = [
            ringB[0:TQR, 1536:2048],
            ringB[0:TQR, 512:1024],
            ringB[0:TQR, 1536:2048],
            ringB[0:TQR, 512:1024],
        ]

        def att_view(i, sc):
            # [32,128] bf16 tiles in ringA banks 2 (sc even) / 3 (sc odd)
            base = 1024 + (sc % 2) * 512 + (i % 2) * 128 + (sc // 2) * 64
            return ringA[:, base : base + 64].bitcast(BF16)

        sem = lambda name: es.enter_context(nc.semaphore(name))
        # per-input-DMA sems (completion order across DMAs is not guaranteed)
        s_q = sem("s_q")          # query (16)
        s_v = [sem(f"s_v{i}") for i in range(SC)]
        s_id = sem("s_id")        # identf+identb (32)
        s_w1 = sem("s_w1")        # (64)
        s_w2 = sem("s_w2")        # (64)
        s_scl = sem("s_scl")      # (16)
        s_trans = sem("s_trans")  # 20 PE transposes
        s_proj = sem("s_proj")    # 4 projection groups (k0,k1,q0,q1)
        s_evac = sem("s_evac")    # 7 DVE evac batches
        s_add = sem("s_add")      # 16 (per tb)
        s_tanh = sem("s_tanh")    # 16
        s_mv = sem("s_mv")        # 16
        s_exp = sem("s_exp")      # 4
        s_sums = sem("s_sums")    # 4
        s_ctxs = sem("s_ctxs")    # 4
        s_att = sem("s_att")      # 16 (4 per quarter)
        s_o = [sem(f"s_o{i}") for i in range(4)]  # 5 each (ctx + 4 attn)
        s_dout = sem("s_dout")    # 128 (8 output DMAs)

        def phase2_pe(tensor, i):
            E = E_Q[i]
            tensor.wait_ge(s_exp, i + 1)
            if i >= 2:
                tensor.wait_ge(s_o[i - 2], 1)  # sums/ctx bank consumers done
            for sc in range(SC):
                ins = tensor.matmul(
                    out=sums_Q[i],
                    lhsT=E[:, sc * TQR : (sc + 1) * TQR],
                    rhs=ones_bf[:, 0:1],
                    start=(sc == 0),
                    stop=(sc == SC - 1),
                )
            ins.then_inc(s_sums, 1)
            for sc in range(SC):
                ins = tensor.matmul(
                    out=ctxp_Q[i],
                    lhsT=E[:, sc * TQR : (sc + 1) * TQR],
                    rhs=v_bf[:, sc * DV : (sc + 1) * DV],
                    start=(sc == 0),
                    stop=(sc == SC - 1),
                )
            ins.then_inc(s_ctxs, 1)
            if i >= 1:
                tensor.wait_ge(s_o[i - 1], 5)  # b2/b3 readers of Q(i-1) done
            if i >= 2:
                tensor.wait_ge(s_o[i - 2], 5)  # region reuse of Q(i-2)
            for sc in range(SC):
                if sc >= 2:
                    tensor.wait_ge(s_o[i], sc)  # own-quarter b2/b3 readers
                tensor.transpose(
                    out=att_view(i, sc)[0:TQR, :],
                    in_=E[:, sc * TQR : (sc + 1) * TQR],
                    identity=ident_bf[:, :],
                ).then_inc(s_att, 1)

        def epilogue_dve(vector, i):
            vector.wait_ge(s_sums, i + 1)
            vector.reciprocal(out=r_Q[i][0:TQR, :], in_=sums_Q[i])
            vector.wait_ge(s_ctxs, i + 1)
            vector.tensor_scalar_mul(
                out=ctx_Q[i][0:TQR, :], in0=ctxp_Q[i], scalar1=r_Q[i][0:TQR, 0:1]
            ).then_inc(s_o[i], 1)
            for sc in range(SC):
                vector.wait_ge(s_att, 4 * i + sc + 1)
                vector.tensor_scalar_mul(
                    out=attn_Q[i][0:TQR, sc * 128 : (sc + 1) * 128],
                    in0=att_view(i, sc)[0:TQR, :],
                    scalar1=r_Q[i][0:TQR, 0:1],
                ).then_inc(s_o[i], 1)

        with nc.Block() as block:

            @block.sync
            def _(sync):
                for sc in range(2):
                    sync.dma_start(
                        out=v_sb[:, sc * DV : (sc + 1) * DV],
                        in_=v_ext[sc * 128 : (sc + 1) * 128, :],
                    ).then_inc(s_v[sc], 16)
                sync.dma_start(out=q_sb[:, :], in_=q_ext[:, :]).then_inc(s_q, 16)
                for i in range(4):
                    sync.wait_ge(s_o[i], 1)
                    sync.dma_start(
                        out=ctx_ext[i * TQR : (i + 1) * TQR, :],
                        in_=ctx_Q[i][0:TQR, :],
                    ).then_inc(s_dout, 16)
                    sync.wait_ge(s_o[i], 5)
                    sync.dma_start(
                        out=attn_ext[i * TQR : (i + 1) * TQR, :],
                        in_=attn_Q[i][0:TQR, :],
                    ).then_inc(s_dout, 16)
                sync.wait_ge(s_dout, 128)

            @block.gpsimd
            def _(gpsimd):
                for dc in range(DC):
                    gpsimd.dma_start(
                        out=w2_sb[:, dc * U : (dc + 1) * U],
                        in_=w2_ext[dc * 128 : (dc + 1) * 128, :],
                    ).then_inc(s_w2, 16)
                for dc in range(DC):
                    gpsimd.dma_start(
                        out=w1_sb[:, dc * U : (dc + 1) * U],
                        in_=w1_ext[dc * 128 : (dc + 1) * 128, :],
                    ).then_inc(s_w1, 16)
                gpsimd.dma_start(out=scale_sb[:, :], in_=scale_ext[:, :]).then_inc(
                    s_scl, 16
                )

            @block.scalar
            def _(scalar):
                scalar.dma_start(out=ident_f[:, :], in_=idf_ext[:, :]).then_inc(
                    s_id, 16
                )
                for sc in range(2, 4):
                    scalar.dma_start(
                        out=v_sb[:, sc * DV : (sc + 1) * DV],
                        in_=v_ext[sc * 128 : (sc + 1) * 128, :],
                    ).then_inc(s_v[sc], 16)
                scalar.dma_start(out=ident_bf[:, :], in_=idb_ext[:, :]).then_inc(
                    s_id, 16
                )
                # phase 1: tanh stream, exp of quarter i after tanh of tb 4i+4
                for tb in range(NTB):
                    scalar.wait_ge(s_add, tb + 1)
                    if tb >= 2:
                        scalar.wait_ge(s_mv, tb - 1)
                    scalar.activation(
                        out=Ts[tb % 2][:, :], in_=Xs[tb % 2][:, :], func=AF.Tanh
                    ).then_inc(s_tanh, 1)
                    if tb in (4, 8, 12):
                        i = tb // 4 - 1
                        scalar.wait_ge(s_mv, tb)
                        scalar.activation(
                            out=E_Q[i][:, :],
                            in_=ringA[:, (i % 2) * 512 + (i // 2) * 128 :][:, 0:128],
                            func=AF.Exp,
                        ).then_inc(s_exp, 1)
                scalar.wait_ge(s_mv, NTB)
                scalar.activation(
                    out=E_Q[3][:, :], in_=ringA[:, 640:768], func=AF.Exp
                ).then_inc(s_exp, 1)

            @block.vector
            def _(vector):
                vector.memset(ones_bf[:, :], 1.0)
                # evac batches 1-4: vT sc (rings A,B,A,B), 5: qT (ring A)
                vTv = vT_sb[:, :].rearrange("p (dc s) -> p dc s", dc=DC)
                rA3 = ringA[:, :].rearrange("p (b x) -> p b x", b=4)
                rB3 = ringB[:, :].rearrange("p (b x) -> p b x", b=4)
                for sc in range(SC):
                    ring3 = rA3 if sc % 2 == 0 else rB3
                    vector.wait_ge(s_trans, 4 * (sc + 1))
                    vector.tensor_copy(
                        out=vTv[:, :, sc * 128 : (sc + 1) * 128],
                        in_=ring3[:, :, 0:128],
                    ).then_inc(s_evac, 1)
                vector.wait_ge(s_trans, 20)
                vector.tensor_copy(out=qT_sb[:, :], in_=rA3[:, :, 0:128]).then_inc(
                    s_evac, 1
                )
                # evac batch 6: k_bf (ring B cols 1024:2048, cast to bf16)
                vector.wait_ge(s_proj, 2)
                vector.tensor_copy(out=k_bf[:, :], in_=ringB[:, 1024:2048]).then_inc(
                    s_evac, 1
                )
                # evac batch 7: q_f (ring B blocks 0,1)
                vector.wait_ge(s_proj, 4)
                vector.tensor_copy(out=q_f[:, :], in_=rB3[:, 0:2, 0:128]).then_inc(
                    s_evac, 1
                )
                # casts
                vector.wait_ge(s_scl, 16)
                vector.tensor_copy(out=scale_bf[:, :], in_=scale_sb[:, :])
                vector.tensor_copy(out=v_bf[:, :], in_=v_sb[:, :])
                # phase 1 adds with quarter epilogues after tb 5, 9, 13
                for tb in range(NTB):
                    buf = Xs[tb % 2]
                    if tb >= 2:
                        vector.wait_ge(s_tanh, tb - 1)
                    for tl in range(TB):
                        t = tb * TB + tl
                        for uc in range(UC):
                            ins = vector.tensor_scalar_add(
                                out=buf[
                                    :, (tl * UC + uc) * TK : (tl * UC + uc + 1) * TK
                                ],
                                in0=k_bf[:, uc * TK : (uc + 1) * TK],
                                scalar1=q_f[:, uc * 128 + t : uc * 128 + t + 1],
                            )
                    ins.then_inc(s_add, 1)
                    if tb in (5, 9, 13):
                        epilogue_dve(vector, tb // 4 - 1)
                epilogue_dve(vector, 3)

            @block.tensor
            def _(tensor):
                tensor.wait_ge(s_id, 32)  # both identities
                # transpose groups: sc0->A, sc1->B, sc2->A, sc3->B, q->A
                for g in range(5):
                    tensor.wait_ge(s_q if g == 4 else s_v[g], 16)
                    if g >= 2:
                        tensor.wait_ge(s_evac, g - 1)
                    ring = ringA if g % 2 == 0 else ringB
                    for dc in range(DC):
                        src = (
                            q_sb[:, dc * 128 : (dc + 1) * 128]
                            if g == 4
                            else v_sb[:, g * DV + dc * 128 : g * DV + dc * 128 + 128]
                        )
                        tensor.transpose(
                            out=ring[:, dc * 512 : dc * 512 + 128],
                            in_=src,
                            identity=ident_f[:, :],
                        ).then_inc(s_trans, 1)
                # projections (fp32): k first (it gates the adds via k_bf)
                tensor.wait_ge(s_w2, 64)
                tensor.wait_ge(s_evac, 4)   # vT complete + ring B free
                for uc in range(UC):
                    for dc in range(DC):
                        ins = tensor.matmul(
                            out=k_ps[uc],
                            lhsT=w2_sb[:, dc * U + uc * 128 : dc * U + uc * 128 + 128],
                            rhs=vT_sb[:, dc * TK : (dc + 1) * TK],
                            start=(dc == 0),
                            stop=(dc == DC - 1),
                        )
                    ins.then_inc(s_proj, 1)
                tensor.wait_ge(s_w1, 64)
                tensor.wait_ge(s_evac, 5)   # qT done
                for uc in range(UC):
                    for dc in range(DC):
                        ins = tensor.matmul(
                            out=q_ps[uc],
                            lhsT=w1_sb[:, dc * U + uc * 128 : dc * U + uc * 128 + 128],
                            rhs=qT_sb[:, dc * 128 : (dc + 1) * 128],
                            start=(dc == 0),
                            stop=(dc == DC - 1),
                        )
                    ins.then_inc(s_proj, 1)
                # phase 1: score matvecs; quarter phase-2 after tb 4, 8, 12
                for tb in range(NTB):
                    tensor.wait_ge(s_tanh, tb + 1)
                    Tt = Ts[tb % 2]
                    for tl in range(TB):
                        t = tb * TB + tl
                        qi, tc = t // TQR, t % TQR
                        col = (qi % 2) * 512 + (qi // 2) * 128
                        for sc in range(SC):
                            for uc in range(UC):
                                base = (tl * UC + uc) * TK + sc * 128
                                ins = tensor.matmul(
                                    out=ringA[:, col + sc * TQR + tc :][:, 0:1],
                                    lhsT=Tt[:, base : base + 128],
                                    rhs=scale_bf[:, uc : uc + 1],
                                    start=(uc == 0),
                                    stop=(uc == UC - 1),
                                )
                    ins.then_inc(s_mv, 1)
                    if tb in (4, 8, 12):
                        phase2_pe(tensor, tb // 4 - 1)
                phase2_pe(tensor, 3)

    return nc


_NC = None


def _get_nc() -> bass.Bass:
    global _NC
    if _NC is None:
        _NC = build_bass()
    return _NC


_EYE_F = None
_EYE_B = None


def make_in_maps(query, value, W1, W2, scale):
    global _EYE_F, _EYE_B
    if _EYE_F is None:
        import ml_dtypes

        _EYE_F = np.eye(128, dtype=np.float32)
        _EYE_B = np.eye(128).astype(ml_dtypes.bfloat16)
    query = np.asarray(query, dtype=np.float32)
    value = np.asarray(value, dtype=np.float32)
    W1 = np.ascontiguousarray(np.asarray(W1, dtype=np.float32))
    W2 = np.ascontiguousarray(np.asarray(W2, dtype=np.float32))
    scale_t = np.ascontiguousarray(
        np.asarray(scale, dtype=np.float32).reshape(UC, 128).T
    )
    in_maps = []
    for c in range(N_CORES):
        b, th = c // 2, c % 2
        in_maps.append(
            {
                "query": np.ascontiguousarray(
                    query[b, th * T_ROWS : (th + 1) * T_ROWS, :]
                ),
                "value": np.ascontiguousarray(value[b]),
                "W1": W1,
                "W2": W2,
                "scale": scale_t,
                "identf": _EYE_F,
                "identb": _EYE_B,
            }
        )
    return in_maps


def assemble(results):
    context = np.empty((B, TQ, DV), dtype=np.float32)
    attn = np.empty((B, TQ, TK), dtype=np.float32)
    for c in range(N_CORES):
        b, th = c // 2, c % 2
        context[b, th * T_ROWS : (th + 1) * T_ROWS, :] = results[c]["context"]
        attn[b, th * T_ROWS : (th + 1) * T_ROWS, :] = results[c]["attn"]
    return context, attn


def kernel(query, value, W1, W2, scale):
    nc = _get_nc()
    in_maps = make_in_maps(query, value, W1, W2, scale)
    res = run_bass_kernel_spmd(nc, in_maps, core_ids=list(range(N_CORES)))
    return assemble(res.results)


# revision 16
# speedup vs baseline: 1.1204x; 1.1178x over previous
"""Additive (Bahdanau) attention on 8 TRN2 NeuronCores (raw Bass).

Reference math (B=4, Tq=256, Tk=512, Dq=Dv=512, U=256):
    q = query @ W1                      [B,Tq,U]
    k = value @ W2                      [B,Tk,U]
    scores[b,t,s] = sum_u scale[u] * tanh(q[b,t,u] + k[b,s,u])
    attn = softmax(scores, axis=-1)     [B,Tq,Tk]
    context = attn @ value              [B,Tq,Dv]
    returns (context, attn)

Sharding: (b, tq-half) -> 8 cores, 128 query rows each; Tk stays local so
there are no collectives.  Per-core dataflow keeps U on partitions for the
big [t,s,u] stage:
    DVE:  X[u, (t,s)] = k[u,s] + q[u,t]   (tensor_scalar add, bf16 4x mode)
    ACT:  T = tanh(X)                     (one big activation per t-block)
    PE :  scoresT[s,t] = sum_u scale[u] T[u,s]   (per-t matvecs, T stationary)
    ACT:  E = exp(scoresT)                (softmax without max: |scores|<~13)
    PE :  sums[t] = E.T @ 1, ctx_raw = E.T @ value, attnT = transpose(E)
    DVE:  r = 1/sums; outputs scaled by r (per-partition scalar)

Engineering notes:
  - this walrus allows only ONE attached sync-wait per instruction, so all
    waits are standalone wait_ge instructions per engine (raw bass).
  - per-input-DMA semaphores: HWDGE completions are NOT FIFO across DMAs.
  - the host passes PRE-TRANSPOSED bf16 copies (queryT, valueT) plus bf16
    weights - no on-chip transposes of the inputs, half the DMA bytes, and
    the k projection starts as soon as valueT chunks land (~8us startup).
  - scores land in PSUM quarter tiles (t in blocks of 32); softmax, context,
    attn-transpose and output DMA of quarter i run interleaved under the
    tanh stream of later t-blocks, so only the last quarter is in the tail.
    DVE epilogue work is split across two t-block slots to avoid starving
    the add stream that feeds ACT (the critical engine).
"""

from contextlib import ExitStack

import numpy as np

import concourse.bass as bass
import concourse.mybir as mybir
from concourse.bass_utils import run_bass_kernel_spmd

F32 = mybir.dt.float32
BF16 = mybir.dt.bfloat16
AF = mybir.ActivationFunctionType

N_CORES = 8
B, TQ, TK, DQ, DV, U = 4, 256, 512, 512, 512, 256
T_ROWS = 128          # query rows per core
TQR = 32              # t-quarter size
UC = U // 128         # u chunks (2)
DC = DQ // 128        # d chunks (4)
SC = TK // 128        # s chunks (4)
TB = 8                # t-block size for the tanh pipeline
NTB = T_ROWS // TB    # 16
XFREE = UC * TB * TK  # 8192 free elems per X/T buffer


def build_bass() -> bass.Bass:
    nc = bass.Bass()
    qt_ext = nc.declare_dram_parameter("queryT", [DQ, T_ROWS], BF16, isOutput=False)
    vt_ext = nc.declare_dram_parameter("valueT", [DV, TK], BF16, isOutput=False)
    vb_ext = nc.declare_dram_parameter("valuebf", [TK, DV], BF16, isOutput=False)
    w1_ext = nc.declare_dram_parameter("W1b", [DQ, U], BF16, isOutput=False)
    w2_ext = nc.declare_dram_parameter("W2b", [DV, U], BF16, isOutput=False)
    scl_ext = nc.declare_dram_parameter("scaleb", [128, UC], BF16, isOutput=False)
    idb_ext = nc.declare_dram_parameter("identb", [128, 128], BF16, isOutput=False)
    ctx_ext = nc.declare_dram_parameter("context", [T_ROWS, DV], F32, isOutput=True)
    attn_ext = nc.declare_dram_parameter("attn", [T_ROWS, TK], F32, isOutput=True)

    es = ExitStack()
    with es:
        _n = [0]

        def sb(shape, dt):
            _n[0] += 1
            return es.enter_context(nc.sbuf_tensor(f"sb{_n[0]}", shape, dt))

        # ---- SBUF ----
        vTb = sb([128, DC * TK], BF16)         # [d_p, (dc, s)]
        qTb = sb([128, DC * 128], BF16)        # [d_p, (dc, t)]
        w1b = sb([128, DC * U], BF16)          # [d_p, (dc, u)]
        w2b = sb([128, DC * U], BF16)
        v_bf = sb([128, SC * DV], BF16)        # [s_p, (sc, d)]
        scale_bf = sb([128, UC], BF16)
        ones_bf = sb([128, 1], BF16)
        ident_bf = sb([128, 128], BF16)
        q_f = sb([128, UC * 128], F32)         # [u_p, (uc, t)]
        k_bf = sb([128, UC * TK], BF16)        # [u_p, (uc, s)]
        X0 = sb([128, XFREE], BF16)
        X1 = sb([128, XFREE], BF16)
        T0 = sb([128, XFREE], BF16)
        T1 = sb([128, XFREE], BF16)
        E_Q = [sb([128, SC * TQR], BF16) for _ in range(4)]  # [s_p, (sc, t32)]
        r_Q = [sb([128, 1], F32) for _ in range(4)]
        ctx_Q = [sb([128, DV], F32) for _ in range(4)]       # rows 0:32 used
        attn_Q = [sb([128, TK], F32) for _ in range(4)]
        Xs, Ts = [X0, X1], [T0, T1]

        # ---- PSUM ----
        ringA = es.enter_context(nc.psum_tensor("ringA", [128, 2048], F32))
        ringB = es.enter_context(nc.psum_tensor("ringB", [128, 2048], F32))
        # ringA: b0/b1 = score quarters, b2/b3 = attnT quarter regions
        #   scores quarter i -> ringA[:, (i%2)*512 + (i//2)*128 : +128]
        #   attnT quarter i  -> 256 f32 cols at 1024 + (i%2)*512 + (i//2)*256
        # ringB: k_ps (b6,b7), q_ps (b4,b5), sums/ctx quarters reuse b4..b7
        k_ps = [ringB[:, 1024:1536], ringB[:, 1536:2048]]
        q_ps = [ringB[:, 0:128], ringB[:, 512:640]]
        sums_Q = [
            ringB[0:TQR, 1024:1025],
            ringB[0:TQR, 0:1],
            ringB[0:TQR, 1024:1025],
            ringB[0:TQR, 0:1],
        ]
        ctxp_Q = [
            ringB[0:TQR, 1536:2048],
            ringB[0:TQR, 512:1024],
            ringB[0:TQR, 1536:2048],
            ringB[0:TQR, 512:1024],
        ]

        def att_base(i):
            return 1024 + (i % 2) * 512 + (i // 2) * 256

        def att_tile(i, sc):
            b = att_base(i)
            return ringA[:, b + sc * 64 : b + (sc + 1) * 64].bitcast(BF16)

        def att_all(i):
            b = att_base(i)
            return ringA[:, b : b + 256].bitcast(BF16)

        sem = lambda name: es.enter_context(nc.semaphore(name))
        s_vt = [sem(f"s_vt{i}") for i in range(DC)]
        s_qt = sem("s_qt")
        s_w1 = sem("s_w1")
        s_w2 = sem("s_w2")
        s_scl = sem("s_scl")
        s_idb = sem("s_idb")
        s_vbf = sem("s_vbf")
        s_proj = sem("s_proj")    # k0,k1,q0,q1
        s_evac = sem("s_evac")    # k_bf, q_f
        s_add = sem("s_add")      # 16
        s_tanh = sem("s_tanh")    # 16
        s_mv = sem("s_mv")        # 16
        s_exp = sem("s_exp")      # 4
        s_sums = sem("s_sums")    # 4
        s_ctxs = sem("s_ctxs")    # 4
        s_att = sem("s_att")      # 16
        s_o = [sem(f"s_o{i}") for i in range(4)]  # ctx=1, attn=2
        s_dout = sem("s_dout")    # 128

        def phase2_pe(tensor, i):
            E = E_Q[i]
            tensor.wait_ge(s_exp, i + 1)
            if i == 0:
                tensor.wait_ge(s_vbf, 16)
                tensor.wait_ge(s_idb, 16)
            if i >= 2:
                tensor.wait_ge(s_o[i - 2], 1)  # sums/ctx bank readers done
            for sc in range(SC):
                ins = tensor.matmul(
                    out=sums_Q[i],
                    lhsT=E[:, sc * TQR : (sc + 1) * TQR],
                    rhs=ones_bf[:, 0:1],
                    start=(sc == 0),
                    stop=(sc == SC - 1),
                )
            ins.then_inc(s_sums, 1)
            for sc in range(SC):
                ins = tensor.matmul(
                    out=ctxp_Q[i],
                    lhsT=E[:, sc * TQR : (sc + 1) * TQR],
                    rhs=v_bf[:, sc * DV : (sc + 1) * DV],
                    start=(sc == 0),
                    stop=(sc == SC - 1),
                )
            ins.then_inc(s_ctxs, 1)
            if i >= 2:
                tensor.wait_ge(s_o[i - 2], 2)  # attnT bank readers done
            for sc in range(SC):
                tensor.transpose(
                    out=att_tile(i, sc)[0:TQR, :],
                    in_=E[:, sc * TQR : (sc + 1) * TQR],
                    identity=ident_bf[:, :],
                ).then_inc(s_att, 1)

        with nc.Block() as block:

            @block.sync
            def _(sync):
                for dc in range(2):
                    sync.dma_start(
                        out=vTb[:, dc * TK : (dc + 1) * TK],
                        in_=vt_ext[dc * 128 : (dc + 1) * 128, :],
                    ).then_inc(s_vt[dc], 16)
                sync.dma_start(
                    out=w2b[:, :].rearrange("p (dc u) -> p dc u", dc=DC),
                    in_=w2_ext[:, :].rearrange("(dc p) u -> p dc u", p=128),
                ).then_inc(s_w2, 16)
                sync.dma_start(
                    out=qTb[:, :].rearrange("p (dc t) -> p dc t", dc=DC),
                    in_=qt_ext[:, :].rearrange("(dc p) t -> p dc t", p=128),
                ).then_inc(s_qt, 16)
                sync.dma_start(
                    out=w1b[:, :].rearrange("p (dc u) -> p dc u", dc=DC),
                    in_=w1_ext[:, :].rearrange("(dc p) u -> p dc u", p=128),
                ).then_inc(s_w1, 16)
                for i in range(4):
                    sync.wait_ge(s_o[i], 1)
                    sync.dma_start(
                        out=ctx_ext[i * TQR : (i + 1) * TQR, :],
                        in_=ctx_Q[i][0:TQR, :],
                    ).then_inc(s_dout, 16)
                    sync.wait_ge(s_o[i], 2)
                    sync.dma_start(
                        out=attn_ext[i * TQR : (i + 1) * TQR, :],
                        in_=attn_Q[i][0:TQR, :],
                    ).then_inc(s_dout, 16)
                sync.wait_ge(s_dout, 128)

            @block.scalar
            def _(scalar):
                for dc in range(2, 4):
                    scalar.dma_start(
                        out=vTb[:, dc * TK : (dc + 1) * TK],
                        in_=vt_ext[dc * 128 : (dc + 1) * 128, :],
                    ).then_inc(s_vt[dc], 16)
                scalar.dma_start(out=scale_bf[:, :], in_=scl_ext[:, :]).then_inc(
                    s_scl, 16
                )
                scalar.dma_start(out=ident_bf[:, :], in_=idb_ext[:, :]).then_inc(
                    s_idb, 16
                )
                scalar.dma_start(
                    out=v_bf[:, :].rearrange("p (sc d) -> p sc d", sc=SC),
                    in_=vb_ext[:, :].rearrange("(sc p) d -> p sc d", p=128),
                ).then_inc(s_vbf, 16)
                # phase 1: tanh stream, exp of quarter i after tanh of tb 4i+4
                for tb in range(NTB):
                    scalar.wait_ge(s_add, tb + 1)
                    if tb >= 2:
                        scalar.wait_ge(s_mv, tb - 1)
                    scalar.activation(
                        out=Ts[tb % 2][:, :], in_=Xs[tb % 2][:, :], func=AF.Tanh
                    ).then_inc(s_tanh, 1)
                    if tb in (4, 8, 12):
                        i = tb // 4 - 1
                        off = (i % 2) * 512 + (i // 2) * 128
                        scalar.wait_ge(s_mv, tb)
                        scalar.activation(
                            out=E_Q[i][:, :],
                            in_=ringA[:, off : off + 128],
                            func=AF.Exp,
                        ).then_inc(s_exp, 1)
                scalar.wait_ge(s_mv, NTB)
                scalar.activation(
                    out=E_Q[3][:, :], in_=ringA[:, 640:768], func=AF.Exp
                ).then_inc(s_exp, 1)

            @block.vector
            def _(vector):
                vector.memset(ones_bf[:, :], 1.0)
                # evacuations: k_bf (cast to bf16), q_f (f32)
                rB3 = ringB[:, :].rearrange("p (b x) -> p b x", b=4)
                vector.wait_ge(s_proj, 2)
                vector.tensor_copy(out=k_bf[:, :], in_=ringB[:, 1024:2048]).then_inc(
                    s_evac, 1
                )
                vector.wait_ge(s_proj, 4)
                vector.tensor_copy(out=q_f[:, :], in_=rB3[:, 0:2, 0:128]).then_inc(
                    s_evac, 1
                )
                # phase 1 adds; epilogue pieces split over tb slots
                for tb in range(NTB):
                    buf = Xs[tb % 2]
                    if tb >= 2:
                        vector.wait_ge(s_tanh, tb - 1)
                    for tl in range(TB):
                        t = tb * TB + tl
                        for uc in range(UC):
                            ins = vector.tensor_scalar_add(
                                out=buf[
                                    :, (tl * UC + uc) * TK : (tl * UC + uc + 1) * TK
                                ],
                                in0=k_bf[:, uc * TK : (uc + 1) * TK],
                                scalar1=q_f[:, uc * 128 + t : uc * 128 + t + 1],
                            )
                    ins.then_inc(s_add, 1)
                    if tb in (5, 9, 13):       # recip + context scale
                        i = tb // 4 - 1
                        vector.wait_ge(s_sums, i + 1)
                        vector.reciprocal(out=r_Q[i][0:TQR, :], in_=sums_Q[i])
                        vector.wait_ge(s_ctxs, i + 1)
                        vector.tensor_scalar_mul(
                            out=ctx_Q[i][0:TQR, :],
                            in0=ctxp_Q[i],
                            scalar1=r_Q[i][0:TQR, 0:1],
                        ).then_inc(s_o[i], 1)
                    if tb in (6, 10, 14):      # attn scale (merged 4 tiles)
                        i = (tb - 2) // 4 - 1
                        vector.wait_ge(s_att, 4 * i + 4)
                        vector.tensor_scalar_mul(
                            out=attn_Q[i][0:TQR, :],
                            in0=att_all(i)[0:TQR, :],
                            scalar1=r_Q[i][0:TQR, 0:1],
                        ).then_inc(s_o[i], 1)
                # quarter 3 epilogue
                vector.wait_ge(s_sums, 4)
                vector.reciprocal(out=r_Q[3][0:TQR, :], in_=sums_Q[3])
                vector.wait_ge(s_ctxs, 4)
                vector.tensor_scalar_mul(
                    out=ctx_Q[3][0:TQR, :], in0=ctxp_Q[3], scalar1=r_Q[3][0:TQR, 0:1]
                ).then_inc(s_o[3], 1)
                vector.wait_ge(s_att, 16)
                vector.tensor_scalar_mul(
                    out=attn_Q[3][0:TQR, :],
                    in0=att_all(3)[0:TQR, :],
                    scalar1=r_Q[3][0:TQR, 0:1],
                ).then_inc(s_o[3], 1)

            @block.tensor
            def _(tensor):
                # k projection (gates the adds via k_bf) - start per vT chunk
                tensor.wait_ge(s_w2, 16)
                for uc in range(UC):
                    for dc in range(DC):
                        if uc == 0:
                            tensor.wait_ge(s_vt[dc], 16)
                        ins = tensor.matmul(
                            out=k_ps[uc],
                            lhsT=w2b[:, dc * U + uc * 128 : dc * U + uc * 128 + 128],
                            rhs=vTb[:, dc * TK : (dc + 1) * TK],
                            start=(dc == 0),
                            stop=(dc == DC - 1),
                        )
                    ins.then_inc(s_proj, 1)
                tensor.wait_ge(s_qt, 16)
                tensor.wait_ge(s_w1, 16)
                for uc in range(UC):
                    for dc in range(DC):
                        ins = tensor.matmul(
                            out=q_ps[uc],
                            lhsT=w1b[:, dc * U + uc * 128 : dc * U + uc * 128 + 128],
                            rhs=qTb[:, dc * 128 : (dc + 1) * 128],
                            start=(dc == 0),
                            stop=(dc == DC - 1),
                        )
                    ins.then_inc(s_proj, 1)
                tensor.wait_ge(s_scl, 16)
                # phase 1: score matvecs; quarter phase-2 after tb 4, 8, 12
                for tb in range(NTB):
                    tensor.wait_ge(s_tanh, tb + 1)
                    Tt = Ts[tb % 2]
                    for tl in range(TB):
                        t = tb * TB + tl
                        qi, tc = t // TQR, t % TQR
                        col = (qi % 2) * 512 + (qi // 2) * 128
                        for sc in range(SC):
                            for uc in range(UC):
                                base = (tl * UC + uc) * TK + sc * 128
                                ins = tensor.matmul(
                                    out=ringA[:, col + sc * TQR + tc :][:, 0:1],
                                    lhsT=Tt[:, base : base + 128],
                                    rhs=scale_bf[:, uc : uc + 1],
                                    start=(uc == 0),
                                    stop=(uc == UC - 1),
                                )
                    ins.then_inc(s_mv, 1)
                    if tb in (4, 8, 12):
                        phase2_pe(tensor, tb // 4 - 1)
                phase2_pe(tensor, 3)

    return nc


_NC = None


def _get_nc() -> bass.Bass:
    global _NC
    if _NC is None:
        _NC = build_bass()
    return _NC


_CONST = None


def make_in_maps(query, value, W1, W2, scale):
    global _CONST
    import ml_dtypes

    bf = ml_dtypes.bfloat16
    if _CONST is None:
        _CONST = {
            "identb": np.eye(128).astype(bf),
        }
    query = np.asarray(query, dtype=np.float32)
    value = np.asarray(value, dtype=np.float32)
    W1b = np.ascontiguousarray(np.asarray(W1, np.float32).astype(bf))
    W2b = np.ascontiguousarray(np.asarray(W2, np.float32).astype(bf))
    scaleb = np.ascontiguousarray(
        np.asarray(scale, np.float32).reshape(UC, 128).T.astype(bf)
    )
    in_maps = []
    for c in range(N_CORES):
        b, th = c // 2, c % 2
        qloc = query[b, th * T_ROWS : (th + 1) * T_ROWS, :]
        vloc = value[b]
        in_maps.append(
            {
                "queryT": np.ascontiguousarray(qloc.T.astype(bf)),
                "valueT": np.ascontiguousarray(vloc.T.astype(bf)),
                "valuebf": np.ascontiguousarray(vloc.astype(bf)),
                "W1b": W1b,
                "W2b": W2b,
                "scaleb": scaleb,
                "identb": _CONST["identb"],
            }
        )
    return in_maps


def assemble(results):
    context = np.empty((B, TQ, DV), dtype=np.float32)
    attn = np.empty((B, TQ, TK), dtype=np.float32)
    for c in range(N_CORES):
        b, th = c // 2, c % 2
        context[b, th * T_ROWS : (th + 1) * T_ROWS, :] = results[c]["context"]
        attn[b, th * T_ROWS : (th + 1) * T_ROWS, :] = results[c]["attn"]
    return context, attn


def kernel(query, value, W1, W2, scale):
    nc = _get_nc()
    in_maps = make_in_maps(query, value, W1, W2, scale)
    res = run_bass_kernel_spmd(nc, in_maps, core_ids=list(range(N_CORES)))
    return assemble(res.results)


# revision 17
# speedup vs baseline: 1.1470x; 1.0237x over previous
"""Additive (Bahdanau) attention on 8 TRN2 NeuronCores (raw Bass).

Reference math (B=4, Tq=256, Tk=512, Dq=Dv=512, U=256):
    q = query @ W1                      [B,Tq,U]
    k = value @ W2                      [B,Tk,U]
    scores[b,t,s] = sum_u scale[u] * tanh(q[b,t,u] + k[b,s,u])
    attn = softmax(scores, axis=-1)     [B,Tq,Tk]
    context = attn @ value              [B,Tq,Dv]
    returns (context, attn)

Sharding: (b, tq-half) -> 8 cores, 128 query rows each; Tk stays local so
there are no collectives.  Per-core dataflow keeps U on partitions for the
big [t,s,u] stage:
    DVE:  X[u, (t,s)] = k[u,s] + q[u,t]   (tensor_scalar add, bf16 4x mode)
    ACT:  T = tanh(X)                     (one big activation per t-block)
    PE :  scoresT[s,t] = sum_u scale[u] T[u,s]   (per-t matvecs, T stationary)
    ACT:  E = exp(scoresT)                (softmax without max: |scores|<~13)
    PE :  sums[t] = E.T @ 1, ctx_raw = E.T @ value, attnT = transpose(E)
    DVE:  r = 1/sums; outputs scaled by r (per-partition scalar)

Engineering notes:
  - this walrus allows only ONE attached sync-wait per instruction, so all
    waits are standalone wait_ge instructions per engine (raw bass).
  - per-input-DMA semaphores: HWDGE completions are NOT FIFO across DMAs.
  - the host passes PRE-TRANSPOSED bf16 copies (queryT, valueT) plus bf16
    weights - no on-chip transposes of the inputs, half the DMA bytes, and
    the k projection starts as soon as valueT chunks land (~8us startup).
  - scores land in PSUM quarter tiles (t in blocks of 32); softmax, context,
    attn-transpose and output DMA of quarter i run interleaved under the
    tanh stream of later t-blocks, so only the last quarter is in the tail.
    DVE epilogue work is split across two t-block slots to avoid starving
    the add stream that feeds ACT (the critical engine).
"""

from contextlib import ExitStack

import numpy as np

import concourse.bass as bass
import concourse.mybir as mybir
from concourse.bass_utils import run_bass_kernel_spmd

F32 = mybir.dt.float32
BF16 = mybir.dt.bfloat16
AF = mybir.ActivationFunctionType

N_CORES = 8
B, TQ, TK, DQ, DV, U = 4, 256, 512, 512, 512, 256
T_ROWS = 128          # query rows per core
TQR = 32              # t-quarter size
UC = U // 128         # u chunks (2)
DC = DQ // 128        # d chunks (4)
SC = TK // 128        # s chunks (4)
TB = 8                # t-block size for the tanh pipeline
NTB = T_ROWS // TB    # 16
XFREE = UC * TB * TK  # 8192 free elems per X/T buffer


def build_bass() -> bass.Bass:
    nc = bass.Bass()
    qt_ext = nc.declare_dram_parameter("queryT", [DQ, T_ROWS], BF16, isOutput=False)
    vt_ext = nc.declare_dram_parameter("valueT", [DV, TK], BF16, isOutput=False)
    vb_ext = nc.declare_dram_parameter("valuebf", [TK, DV], BF16, isOutput=False)
    w1_ext = nc.declare_dram_parameter("W1b", [DQ, U], BF16, isOutput=False)
    w2_ext = nc.declare_dram_parameter("W2b", [DV, U], BF16, isOutput=False)
    scl_ext = nc.declare_dram_parameter("scaleb", [128, UC], BF16, isOutput=False)
    idb_ext = nc.declare_dram_parameter("identb", [128, 128], BF16, isOutput=False)
    ctx_ext = nc.declare_dram_parameter("context", [T_ROWS, DV], F32, isOutput=True)
    attn_ext = nc.declare_dram_parameter("attn", [T_ROWS, TK], F32, isOutput=True)

    es = ExitStack()
    with es:
        _n = [0]

        def sb(shape, dt):
            _n[0] += 1
            return es.enter_context(nc.sbuf_tensor(f"sb{_n[0]}", shape, dt))

        # ---- SBUF ----
        vTb = sb([128, DC * TK], BF16)         # [d_p, (dc, s)]
        qTb = sb([128, DC * 128], BF16)        # [d_p, (dc, t)]
        w1b = sb([128, DC * U], BF16)          # [d_p, (dc, u)]
        w2b = sb([128, DC * U], BF16)
        v_bf = sb([128, SC * DV], BF16)        # [s_p, (sc, d)]
        scale_bf = sb([128, UC], BF16)
        ones_bf = sb([128, 1], BF16)
        ident_bf = sb([128, 128], BF16)
        q_f = sb([128, UC * 128], F32)         # [u_p, (uc, t)]
        k_bf = sb([128, UC * TK], BF16)        # [u_p, (uc, s)]
        X0 = sb([128, XFREE], BF16)
        X1 = sb([128, XFREE], BF16)
        X2 = sb([128, XFREE], BF16)
        T0 = sb([128, XFREE], BF16)
        T1 = sb([128, XFREE], BF16)
        E_Q = [sb([128, SC * TQR], BF16) for _ in range(4)]  # [s_p, (sc, t32)]
        r_Q = [sb([128, 1], F32) for _ in range(4)]
        ctx_Q = [sb([128, DV], F32) for _ in range(4)]       # rows 0:32 used
        attn_Q = [sb([128, TK], F32) for _ in range(4)]
        Xs, Ts = [X0, X1, X2], [T0, T1]
        # tanh segments: (tb, lo_tl, hi_tl); tb0 and tb15 are split in half
        TANH_SEGS = (
            [(0, 0, 4), (0, 4, 8)]
            + [(tb, 0, 8) for tb in range(1, 15)]
            + [(15, 0, 4), (15, 4, 8)]
        )
        # s_add threshold needed by tanh seg k / s_tanh threshold after seg k
        SEG_ADD_WAIT = [1, 2] + [tb + 2 for tb in range(1, 15)] + [17, 17]
        # PE: s_tanh threshold for (tb, tl): tb0 tl<4 ->1, tl>=4 ->2; etc.
        def mv_tanh_thresh(tb, tl):
            if tb == 0:
                return 1 if tl < 4 else 2
            if tb == 15:
                return 17 if tl < 4 else 18
            return tb + 2

        # ---- PSUM ----
        ringA = es.enter_context(nc.psum_tensor("ringA", [128, 2048], F32))
        ringB = es.enter_context(nc.psum_tensor("ringB", [128, 2048], F32))
        # ringA: b0/b1 = score quarters, b2/b3 = attnT quarter regions
        #   scores quarter i -> ringA[:, (i%2)*512 + (i//2)*128 : +128]
        #   attnT quarter i  -> 256 f32 cols at 1024 + (i%2)*512 + (i//2)*256
        # ringB: k_ps (b6,b7), q_ps (b4,b5), sums/ctx quarters reuse b4..b7
        k_ps = [ringB[:, 1024:1536], ringB[:, 1536:2048]]
        q_ps = [ringB[:, 0:128], ringB[:, 512:640]]
        sums_Q = [
            ringB[0:TQR, 1024:1025],
            ringB[0:TQR, 0:1],
            ringB[0:TQR, 1024:1025],
            ringB[0:TQR, 0:1],
        ]
        ctxp_Q = [
            ringB[0:TQR, 1536:2048],
            ringB[0:TQR, 512:1024],
            ringB[0:TQR, 1536:2048],
            ringB[0:TQR, 512:1024],
        ]

        def att_base(i):
            return 1024 + (i % 2) * 512 + (i // 2) * 256

        def att_tile(i, sc):
            b = att_base(i)
            return ringA[:, b + sc * 64 : b + (sc + 1) * 64].bitcast(BF16)

        def att_all(i):
            b = att_base(i)
            return ringA[:, b : b + 256].bitcast(BF16)

        sem = lambda name: es.enter_context(nc.semaphore(name))
        s_vt = [sem(f"s_vt{i}") for i in range(DC)]
        s_qt = sem("s_qt")
        s_w1 = sem("s_w1")
        s_w2 = sem("s_w2")
        s_scl = sem("s_scl")
        s_idb = sem("s_idb")
        s_vbf = sem("s_vbf")
        s_proj = sem("s_proj")    # k0,k1,q0,q1
        s_evac = sem("s_evac")    # k_bf, q_f
        s_add = sem("s_add")      # 16
        s_tanh = sem("s_tanh")    # 16
        s_mv = sem("s_mv")        # 16
        s_exp = sem("s_exp")      # 4
        s_sums = sem("s_sums")    # 4
        s_ctxs = sem("s_ctxs")    # 4
        s_att = sem("s_att")      # 16
        s_o = [sem(f"s_o{i}") for i in range(4)]  # ctx=1, attn=2
        s_dout = sem("s_dout")    # 128

        def phase2_pe(tensor, i):
            E = E_Q[i]
            tensor.wait_ge(s_exp, i + 1)
            if i == 0:
                tensor.wait_ge(s_vbf, 16)
                tensor.wait_ge(s_idb, 16)
            if i >= 2:
                tensor.wait_ge(s_o[i - 2], 1)  # sums/ctx bank readers done
            for sc in range(SC):
                ins = tensor.matmul(
                    out=sums_Q[i],
                    lhsT=E[:, sc * TQR : (sc + 1) * TQR],
                    rhs=ones_bf[:, 0:1],
                    start=(sc == 0),
                    stop=(sc == SC - 1),
                )
            ins.then_inc(s_sums, 1)
            for sc in range(SC):
                ins = tensor.matmul(
                    out=ctxp_Q[i],
                    lhsT=E[:, sc * TQR : (sc + 1) * TQR],
                    rhs=v_bf[:, sc * DV : (sc + 1) * DV],
                    start=(sc == 0),
                    stop=(sc == SC - 1),
                )
            ins.then_inc(s_ctxs, 1)
            if i >= 2:
                tensor.wait_ge(s_o[i - 2], 2)  # attnT bank readers done
            for sc in range(SC):
                tensor.transpose(
                    out=att_tile(i, sc)[0:TQR, :],
                    in_=E[:, sc * TQR : (sc + 1) * TQR],
                    identity=ident_bf[:, :],
                ).then_inc(s_att, 1)

        with nc.Block() as block:

            @block.sync
            def _(sync):
                for dc in range(2):
                    sync.dma_start(
                        out=vTb[:, dc * TK : (dc + 1) * TK],
                        in_=vt_ext[dc * 128 : (dc + 1) * 128, :],
                    ).then_inc(s_vt[dc], 16)
                for i in range(4):
                    sync.wait_ge(s_o[i], 1)
                    sync.dma_start(
                        out=ctx_ext[i * TQR : (i + 1) * TQR, :],
                        in_=ctx_Q[i][0:TQR, :],
                    ).then_inc(s_dout, 16)
                    sync.wait_ge(s_o[i], 2)
                    sync.dma_start(
                        out=attn_ext[i * TQR : (i + 1) * TQR, :],
                        in_=attn_Q[i][0:TQR, :],
                    ).then_inc(s_dout, 16)
                sync.wait_ge(s_dout, 128)

            @block.gpsimd
            def _(gpsimd):
                gpsimd.dma_start(
                    out=qTb[:, :].rearrange("p (dc t) -> p dc t", dc=DC),
                    in_=qt_ext[:, :].rearrange("(dc p) t -> p dc t", p=128),
                ).then_inc(s_qt, 16)
                gpsimd.dma_start(
                    out=w1b[:, :].rearrange("p (dc u) -> p dc u", dc=DC),
                    in_=w1_ext[:, :].rearrange("(dc p) u -> p dc u", p=128),
                ).then_inc(s_w1, 16)
                gpsimd.dma_start(out=scale_bf[:, :], in_=scl_ext[:, :]).then_inc(
                    s_scl, 16
                )
                gpsimd.dma_start(out=ident_bf[:, :], in_=idb_ext[:, :]).then_inc(
                    s_idb, 16
                )
                gpsimd.dma_start(
                    out=v_bf[:, :].rearrange("p (sc d) -> p sc d", sc=SC),
                    in_=vb_ext[:, :].rearrange("(sc p) d -> p sc d", p=128),
                ).then_inc(s_vbf, 16)

            @block.scalar
            def _(scalar):
                scalar.dma_start(
                    out=w2b[:, :].rearrange("p (dc u) -> p dc u", dc=DC),
                    in_=w2_ext[:, :].rearrange("(dc p) u -> p dc u", p=128),
                ).then_inc(s_w2, 16)
                for dc in range(2, 4):
                    scalar.dma_start(
                        out=vTb[:, dc * TK : (dc + 1) * TK],
                        in_=vt_ext[dc * 128 : (dc + 1) * 128, :],
                    ).then_inc(s_vt[dc], 16)
                # phase 1: tanh stream, exp of quarter i after tanh of tb 4i+4
                prev_tb = -1
                for k, (tb, lo, hi) in enumerate(TANH_SEGS):
                    scalar.wait_ge(s_add, SEG_ADD_WAIT[k])
                    if tb != prev_tb and tb >= 2:
                        scalar.wait_ge(s_mv, tb - 1)
                    prev_tb = tb
                    scalar.activation(
                        out=Ts[tb % 2][:, lo * UC * TK : hi * UC * TK],
                        in_=Xs[tb % 3][:, lo * UC * TK : hi * UC * TK],
                        func=AF.Tanh,
                    ).then_inc(s_tanh, 1)
                    if hi == 8 and tb in (4, 8, 12):
                        i = tb // 4 - 1
                        off = (i % 2) * 512 + (i // 2) * 128
                        scalar.wait_ge(s_mv, tb)
                        scalar.activation(
                            out=E_Q[i][:, :],
                            in_=ringA[:, off : off + 128],
                            func=AF.Exp,
                        ).then_inc(s_exp, 1)
                scalar.wait_ge(s_mv, NTB)
                scalar.activation(
                    out=E_Q[3][:, :], in_=ringA[:, 640:768], func=AF.Exp
                ).then_inc(s_exp, 1)

            @block.vector
            def _(vector):
                vector.memset(ones_bf[:, :], 1.0)
                # evacuations: k_bf (cast to bf16), q_f (f32)
                rB3 = ringB[:, :].rearrange("p (b x) -> p b x", b=4)
                vector.wait_ge(s_proj, 2)
                vector.tensor_copy(out=k_bf[:, :], in_=ringB[:, 1024:2048]).then_inc(
                    s_evac, 1
                )
                vector.wait_ge(s_proj, 4)
                vector.tensor_copy(out=q_f[:, :], in_=rB3[:, 0:2, 0:128]).then_inc(
                    s_evac, 1
                )
                # phase 1 adds; epilogue pieces split over tb slots
                for tb in range(NTB):
                    buf = Xs[tb % 3]
                    if tb >= 3:
                        # buffer reused from tb-3: tanh(tb-3) done at s_tanh
                        # >= tb-1 in segmented numbering
                        vector.wait_ge(s_tanh, tb - 1)
                    for tl in range(TB):
                        t = tb * TB + tl
                        for uc in range(UC):
                            ins = vector.tensor_scalar_add(
                                out=buf[
                                    :, (tl * UC + uc) * TK : (tl * UC + uc + 1) * TK
                                ],
                                in0=k_bf[:, uc * TK : (uc + 1) * TK],
                                scalar1=q_f[:, uc * 128 + t : uc * 128 + t + 1],
                            )
                        if tb == 0 and tl == 3:
                            ins.then_inc(s_add, 1)
                    ins.then_inc(s_add, 1)
                    if tb in (5, 9, 13):       # recip + context scale
                        i = tb // 4 - 1
                        vector.wait_ge(s_sums, i + 1)
                        vector.reciprocal(out=r_Q[i][0:TQR, :], in_=sums_Q[i])
                        vector.wait_ge(s_ctxs, i + 1)
                        vector.tensor_scalar_mul(
                            out=ctx_Q[i][0:TQR, :],
                            in0=ctxp_Q[i],
                            scalar1=r_Q[i][0:TQR, 0:1],
                        ).then_inc(s_o[i], 1)
                    if tb in (6, 10, 14):      # attn scale (merged 4 tiles)
                        i = (tb - 2) // 4 - 1
                        vector.wait_ge(s_att, 4 * i + 4)
                        vector.tensor_scalar_mul(
                            out=attn_Q[i][0:TQR, :],
                            in0=att_all(i)[0:TQR, :],
                            scalar1=r_Q[i][0:TQR, 0:1],
                        ).then_inc(s_o[i], 1)
                # quarter 3 epilogue
                vector.wait_ge(s_sums, 4)
                vector.reciprocal(out=r_Q[3][0:TQR, :], in_=sums_Q[3])
                vector.wait_ge(s_ctxs, 4)
                vector.tensor_scalar_mul(
                    out=ctx_Q[3][0:TQR, :], in0=ctxp_Q[3], scalar1=r_Q[3][0:TQR, 0:1]
                ).then_inc(s_o[3], 1)
                vector.wait_ge(s_att, 16)
                vector.tensor_scalar_mul(
                    out=attn_Q[3][0:TQR, :],
                    in0=att_all(3)[0:TQR, :],
                    scalar1=r_Q[3][0:TQR, 0:1],
                ).then_inc(s_o[3], 1)

            @block.tensor
            def _(tensor):
                # k projection (gates the adds via k_bf) - start per vT chunk
                tensor.wait_ge(s_w2, 16)
                for uc in range(UC):
                    for dc in range(DC):
                        if uc == 0:
                            tensor.wait_ge(s_vt[dc], 16)
                        ins = tensor.matmul(
                            out=k_ps[uc],
                            lhsT=w2b[:, dc * U + uc * 128 : dc * U + uc * 128 + 128],
                            rhs=vTb[:, dc * TK : (dc + 1) * TK],
                            start=(dc == 0),
                            stop=(dc == DC - 1),
                        )
                    ins.then_inc(s_proj, 1)
                tensor.wait_ge(s_qt, 16)
                tensor.wait_ge(s_w1, 16)
                for uc in range(UC):
                    for dc in range(DC):
                        ins = tensor.matmul(
                            out=q_ps[uc],
                            lhsT=w1b[:, dc * U + uc * 128 : dc * U + uc * 128 + 128],
                            rhs=qTb[:, dc * 128 : (dc + 1) * 128],
                            start=(dc == 0),
                            stop=(dc == DC - 1),
                        )
                    ins.then_inc(s_proj, 1)
                tensor.wait_ge(s_scl, 16)
                # phase 1: score matvecs; quarter phase-2 after tb 4, 8, 12
                for tb in range(NTB):
                    tensor.wait_ge(s_tanh, mv_tanh_thresh(tb, 0))
                    Tt = Ts[tb % 2]
                    for tl in range(TB):
                        if tb in (0, 15) and tl == 4:
                            tensor.wait_ge(s_tanh, mv_tanh_thresh(tb, 4))
                        t = tb * TB + tl
                        qi, tc = t // TQR, t % TQR
                        col = (qi % 2) * 512 + (qi // 2) * 128
                        for sc in range(SC):
                            for uc in range(UC):
                                base = (tl * UC + uc) * TK + sc * 128
                                ins = tensor.matmul(
                                    out=ringA[:, col + sc * TQR + tc :][:, 0:1],
                                    lhsT=Tt[:, base : base + 128],
                                    rhs=scale_bf[:, uc : uc + 1],
                                    start=(uc == 0),
                                    stop=(uc == UC - 1),
                                )
                    ins.then_inc(s_mv, 1)
                    if tb in (4, 8, 12):
                        phase2_pe(tensor, tb // 4 - 1)
                phase2_pe(tensor, 3)

    return nc


_NC = None


def _get_nc() -> bass.Bass:
    global _NC
    if _NC is None:
        _NC = build_bass()
    return _NC


_CONST = None


def make_in_maps(query, value, W1, W2, scale):
    global _CONST
    import ml_dtypes

    bf = ml_dtypes.bfloat16
    if _CONST is None:
        _CONST = {
            "identb": np.eye(128).astype(bf),
        }
    query = np.asarray(query, dtype=np.float32)
    value = np.asarray(value, dtype=np.float32)
    W1b = np.ascontiguousarray(np.asarray(W1, np.float32).astype(bf))
    W2b = np.ascontiguousarray(np.asarray(W2, np.float32).astype(bf))
    scaleb = np.ascontiguousarray(
        np.asarray(scale, np.float32).reshape(UC, 128).T.astype(bf)
    )
    in_maps = []
    for c in range(N_CORES):
        b, th = c // 2, c % 2
        qloc = query[b, th * T_ROWS : (th + 1) * T_ROWS, :]
        vloc = value[b]
        in_maps.append(
            {
                "queryT": np.ascontiguousarray(qloc.T.astype(bf)),
                "valueT": np.ascontiguousarray(vloc.T.astype(bf)),
                "valuebf": np.ascontiguousarray(vloc.astype(bf)),
                "W1b": W1b,
                "W2b": W2b,
                "scaleb": scaleb,
                "identb": _CONST["identb"],
            }
        )
    return in_maps


def assemble(results):
    context = np.empty((B, TQ, DV), dtype=np.float32)
    attn = np.empty((B, TQ, TK), dtype=np.float32)
    for c in range(N_CORES):
        b, th = c // 2, c % 2
        context[b, th * T_ROWS : (th + 1) * T_ROWS, :] = results[c]["context"]
        attn[b, th * T_ROWS : (th + 1) * T_ROWS, :] = results[c]["attn"]
    return context, attn


def kernel(query, value, W1, W2, scale):
    nc = _get_nc()
    in_maps = make_in_maps(query, value, W1, W2, scale)
    res = run_bass_kernel_spmd(nc, in_maps, core_ids=list(range(N_CORES)))
    return assemble(res.results)


# revision 21
# speedup vs baseline: 1.1772x; 1.0264x over previous
"""Additive (Bahdanau) attention on 8 TRN2 NeuronCores (raw Bass).

Reference math (B=4, Tq=256, Tk=512, Dq=Dv=512, U=256):
    q = query @ W1                      [B,Tq,U]
    k = value @ W2                      [B,Tk,U]
    scores[b,t,s] = sum_u scale[u] * tanh(q[b,t,u] + k[b,s,u])
    attn = softmax(scores, axis=-1)     [B,Tq,Tk]
    context = attn @ value              [B,Tq,Dv]
    returns (context, attn)

Sharding: (b, tq-half) -> 8 cores, 128 query rows each; Tk stays local so
there are no collectives.  Per-core dataflow keeps U on partitions for the
big [t,s,u] stage:
    DVE:  X[u, (t,s)] = k[u,s] + q[u,t]   (tensor_scalar add, bf16 4x mode)
    ACT:  T = tanh(X)                     (one big activation per t-block)
    PE :  scoresT[s,t] = sum_u scale[u] T[u,s]   (per-t matvecs, T stationary)
    ACT:  E = exp(scoresT)                (softmax without max: |scores|<~13)
    PE :  sums[t] = E.T @ 1, ctx_raw = E.T @ value, attnT = transpose(E)
    DVE:  r = 1/sums; outputs scaled by r (per-partition scalar)

Engineering notes:
  - this walrus allows only ONE attached sync-wait per instruction, so all
    waits are standalone wait_ge instructions per engine (raw bass).
  - per-input-DMA semaphores: HWDGE completions are NOT FIFO across DMAs.
  - the host passes PRE-TRANSPOSED bf16 operands (queryT, valueT, bf16
    weights/value) - no on-chip input transposes and half the DMA bytes.
    critical loads are spread over four DMA paths (sync+scalar HWDGE,
    gpsimd+vector SWDGE) so the k projection starts ~10us in.
  - the DVE's scalar operand (tensor_scalar/activation bias) is prefetched
    by the sequencer BEFORE the previous op's writes drain, so a value
    produced by the immediately-preceding DVE op needs a drain or an
    intervening op before it is consumed as a scalar.
  - softmax/context/attn run in four UNEVEN t-groups (40/40/32/16 rows):
    groups 0-2 are processed under the tanh stream of later t-blocks and
    only the tiny 16-row group 3 remains in the tail.
"""

from contextlib import ExitStack

import numpy as np

import concourse.bass as bass
import concourse.mybir as mybir
from concourse.bass_utils import run_bass_kernel_spmd

F32 = mybir.dt.float32
BF16 = mybir.dt.bfloat16
AF = mybir.ActivationFunctionType

N_CORES = 8
B, TQ, TK, DQ, DV, U = 4, 256, 512, 512, 512, 256
T_ROWS = 128          # query rows per core
UC = U // 128         # u chunks (2)
DC = DQ // 128        # d chunks (4)
SC = TK // 128        # s chunks (4)
TB = 8                # t-block size for the tanh pipeline
NTB = T_ROWS // TB    # 16
XFREE = UC * TB * TK  # 8192 free elems per X/T buffer

# phase-2 groups: (t0, n_rows), score-tile base col, attnT base col,
# slots: exp after tanh tb / pe after mv tb / recip after adds tb /
#        muls after adds tb  (None = after the loop)
GROUPS = [
    dict(t0=0, n=40, col=0, att=1024, exp=5, pe=5, rc=9, mul=10),
    dict(t0=40, n=40, col=512, att=1536, exp=10, pe=10, rc=14, mul=15),
    dict(t0=80, n=32, col=160, att=1280, exp=14, pe=14, rc=None, mul=None),
    dict(t0=112, n=16, col=672, att=1792, exp=None, pe=None, rc=None, mul=None),
]


def grp_of(t):
    for gi, g in enumerate(GROUPS):
        if g["t0"] <= t < g["t0"] + g["n"]:
            return gi, g
    raise AssertionError


def build_bass() -> bass.Bass:
    nc = bass.Bass()
    qt_ext = nc.declare_dram_parameter("queryT", [DQ, T_ROWS], BF16, isOutput=False)
    vt_ext = nc.declare_dram_parameter("valueT", [DV, TK], BF16, isOutput=False)
    vb_ext = nc.declare_dram_parameter("valuebf", [TK, DV], BF16, isOutput=False)
    w1_ext = nc.declare_dram_parameter("W1b", [DQ, U], BF16, isOutput=False)
    w2_ext = nc.declare_dram_parameter("W2b", [DV, U], BF16, isOutput=False)
    scl_ext = nc.declare_dram_parameter("scaleb", [128, UC], BF16, isOutput=False)
    idb_ext = nc.declare_dram_parameter("identb", [128, 128], BF16, isOutput=False)
    ctx_ext = nc.declare_dram_parameter("context", [T_ROWS, DV], F32, isOutput=True)
    attn_ext = nc.declare_dram_parameter("attn", [T_ROWS, TK], F32, isOutput=True)

    es = ExitStack()
    with es:
        _n = [0]

        def sb(shape, dt):
            _n[0] += 1
            return es.enter_context(nc.sbuf_tensor(f"sb{_n[0]}", shape, dt))

        # ---- SBUF ----
        vTb = sb([128, DC * TK], BF16)         # [d_p, (dc, s)]
        qTb = sb([128, DC * 128], BF16)        # [d_p, (dc, t)]
        w1b = sb([128, DC * U], BF16)          # [d_p, (dc, u)]
        w2b = sb([128, DC * U], BF16)
        v_bf = sb([128, SC * DV], BF16)        # [s_p, (sc, d)]
        scale_bf = sb([128, UC], BF16)
        ones_bf = sb([128, 1], BF16)
        ident_bf = sb([128, 128], BF16)
        q_f = sb([128, UC * 128], F32)         # [u_p, (uc, t)]
        k_bf = sb([128, UC * TK], BF16)        # [u_p, (uc, s)]
        X0 = sb([128, XFREE], BF16)
        X1 = sb([128, XFREE], BF16)
        X2 = sb([128, XFREE], BF16)
        T0 = sb([128, XFREE], BF16)
        T1 = sb([128, XFREE], BF16)
        E_G = [sb([128, SC * g["n"]], BF16) for g in GROUPS]  # [s_p, (sc, t)]
        r_G = [sb([128, 1], F32) for _ in GROUPS]
        ctx_G = [sb([128, DV], F32) for _ in GROUPS]          # rows 0:n used
        attn_G = [sb([128, TK], F32) for _ in GROUPS]
        Xs, Ts = [X0, X1, X2], [T0, T1]

        # tanh segments: (tb, lo_tl, hi_tl); tb0 and tb15 are split in half
        TANH_SEGS = (
            [(0, 0, 4), (0, 4, 8)]
            + [(tb, 0, 8) for tb in range(1, 15)]
            + [(15, 0, 4), (15, 4, 8)]
        )
        SEG_ADD_WAIT = [1, 2] + [tb + 2 for tb in range(1, 15)] + [17, 17]

        def mv_tanh_thresh(tb, tl):
            if tb == 0:
                return 1 if tl < 4 else 2
            if tb == 15:
                return 17 if tl < 4 else 18
            return tb + 2

        # ---- PSUM ----
        ringA = es.enter_context(nc.psum_tensor("ringA", [128, 2048], F32))
        ringB = es.enter_context(nc.psum_tensor("ringB", [128, 2048], F32))
        k_ps = [ringB[:, 1024:1536], ringB[:, 1536:2048]]
        q_ps = [ringB[:, 0:128], ringB[:, 512:640]]
        # sums/ctx banks alternate b6/b7 and b4/b5 per group
        sums_G = [
            ringB[0 : GROUPS[i]["n"], 1024 + (i % 2) * -1024 :][:, 0:1]
            for i in range(4)
        ]
        sums_G = [
            ringB[0 : GROUPS[0]["n"], 1024:1025],
            ringB[0 : GROUPS[1]["n"], 0:1],
            ringB[0 : GROUPS[2]["n"], 1024:1025],
            ringB[0 : GROUPS[3]["n"], 0:1],
        ]
        ctxp_G = [
            ringB[0 : GROUPS[0]["n"], 1536:2048],
            ringB[0 : GROUPS[1]["n"], 512:1024],
            ringB[0 : GROUPS[2]["n"], 1536:2048],
            ringB[0 : GROUPS[3]["n"], 512:1024],
        ]

        def att_tile(i, sc):
            b = GROUPS[i]["att"]
            return ringA[:, b + sc * 64 : b + (sc + 1) * 64].bitcast(BF16)

        def att_all(i):
            b = GROUPS[i]["att"]
            return ringA[:, b : b + 256].bitcast(BF16)

        sem = lambda name: es.enter_context(nc.semaphore(name))
        s_vt = [sem(f"s_vt{i}") for i in range(DC)]
        s_qt = sem("s_qt")
        s_w1 = sem("s_w1")
        s_w2 = sem("s_w2")
        s_scl = sem("s_scl")
        s_idb = sem("s_idb")
        s_vbf = sem("s_vbf")
        s_proj = sem("s_proj")    # k0,k1,q0,q1
        s_evac = sem("s_evac")    # q_f, k_bf
        s_add = sem("s_add")      # 17 (tb0 split)
        s_tanh = sem("s_tanh")    # 18 (tb0/tb15 split)
        s_mv = sem("s_mv")        # 16
        s_exp = sem("s_exp")      # 4
        s_sums = sem("s_sums")    # 4
        s_ctxs = sem("s_ctxs")    # 4
        s_att = sem("s_att")      # 16
        s_o = [sem(f"s_o{i}") for i in range(4)]  # ctx=1, attn=2
        s_dout = sem("s_dout")    # 128

        def phase2_pe(tensor, i):
            g = GROUPS[i]
            n = g["n"]
            E = E_G[i]
            tensor.wait_ge(s_exp, i + 1)
            if i == 0:
                tensor.wait_ge(s_vbf, 16)
                tensor.wait_ge(s_idb, 16)
            if i >= 2:
                tensor.wait_ge(s_o[i - 2], 1)  # sums/ctx bank readers done
            for sc in range(SC):
                ins = tensor.matmul(
                    out=sums_G[i],
                    lhsT=E[:, sc * n : (sc + 1) * n],
                    rhs=ones_bf[:, 0:1],
                    start=(sc == 0),
                    stop=(sc == SC - 1),
                )
            ins.then_inc(s_sums, 1)
            for sc in range(SC):
                ins = tensor.matmul(
                    out=ctxp_G[i],
                    lhsT=E[:, sc * n : (sc + 1) * n],
                    rhs=v_bf[:, sc * DV : (sc + 1) * DV],
                    start=(sc == 0),
                    stop=(sc == SC - 1),
                )
            ins.then_inc(s_ctxs, 1)
            if i >= 2:
                tensor.wait_ge(s_o[i - 2], 2)  # attnT bank readers done
            for sc in range(SC):
                tensor.transpose(
                    out=att_tile(i, sc)[0:n, :],
                    in_=E[:, sc * n : (sc + 1) * n],
                    identity=ident_bf[:, :],
                ).then_inc(s_att, 1)

        def rc_dve(vector, i):
            # reciprocal in its own slot + drain: r is consumed as a scalar
            # operand later and scalar reads bypass the DVE pipe
            n = GROUPS[i]["n"]
            vector.wait_ge(s_sums, i + 1)
            vector.reciprocal(out=r_G[i][0:n, :], in_=sums_G[i])
            vector.drain()

        def mul_dve(vector, i):
            n = GROUPS[i]["n"]
            vector.wait_ge(s_ctxs, i + 1)
            vector.tensor_scalar_mul(
                out=ctx_G[i][0:n, :], in0=ctxp_G[i], scalar1=r_G[i][0:n, 0:1]
            ).then_inc(s_o[i], 1)
            vector.wait_ge(s_att, 4 * i + 4)
            vector.tensor_scalar_mul(
                out=attn_G[i][0:n, :],
                in0=att_all(i)[0:n, :],
                scalar1=r_G[i][0:n, 0:1],
            ).then_inc(s_o[i], 1)

        with nc.Block() as block:

            @block.sync
            def _(sync):
                sync.dma_start(
                    out=vTb[:, 0:TK], in_=vt_ext[0:128, :]
                ).then_inc(s_vt[0], 16)
                sync.dma_start(
                    out=qTb[:, :].rearrange("p (dc t) -> p dc t", dc=DC),
                    in_=qt_ext[:, :].rearrange("(dc p) t -> p dc t", p=128),
                ).then_inc(s_qt, 16)
                sync.dma_start(
                    out=w1b[:, :].rearrange("p (dc u) -> p dc u", dc=DC),
                    in_=w1_ext[:, :].rearrange("(dc p) u -> p dc u", p=128),
                ).then_inc(s_w1, 16)
                for i in range(4):
                    g = GROUPS[i]
                    sync.wait_ge(s_o[i], 1)
                    sync.dma_start(
                        out=ctx_ext[g["t0"] : g["t0"] + g["n"], :],
                        in_=ctx_G[i][0 : g["n"], :],
                    ).then_inc(s_dout, 16)
                    sync.wait_ge(s_o[i], 2)
                    sync.dma_start(
                        out=attn_ext[g["t0"] : g["t0"] + g["n"], :],
                        in_=attn_G[i][0 : g["n"], :],
                    ).then_inc(s_dout, 16)
                sync.wait_ge(s_dout, 128)

            @block.scalar
            def _(scalar):
                scalar.dma_start(
                    out=w2b[:, :].rearrange("p (dc u) -> p dc u", dc=DC),
                    in_=w2_ext[:, :].rearrange("(dc p) u -> p dc u", p=128),
                ).then_inc(s_w2, 16)
                scalar.dma_start(
                    out=vTb[:, TK : 2 * TK], in_=vt_ext[128:256, :]
                ).then_inc(s_vt[1], 16)
                scalar.dma_start(
                    out=vTb[:, 3 * TK : 4 * TK], in_=vt_ext[384:512, :]
                ).then_inc(s_vt[3], 16)
                # phase 1: tanh stream with group exps woven in
                prev_tb = -1
                exp_at = {g["exp"]: i for i, g in enumerate(GROUPS) if g["exp"]}
                for k, (tb, lo, hi) in enumerate(TANH_SEGS):
                    scalar.wait_ge(s_add, SEG_ADD_WAIT[k])
                    if tb != prev_tb and tb >= 2:
                        scalar.wait_ge(s_mv, tb - 1)
                    prev_tb = tb
                    scalar.activation(
                        out=Ts[tb % 2][:, lo * UC * TK : hi * UC * TK],
                        in_=Xs[tb % 3][:, lo * UC * TK : hi * UC * TK],
                        func=AF.Tanh,
                    ).then_inc(s_tanh, 1)
                    if hi == 8 and tb in exp_at:
                        i = exp_at[tb]
                        g = GROUPS[i]
                        scalar.wait_ge(s_mv, tb)
                        scalar.activation(
                            out=E_G[i][:, :],
                            in_=ringA[:, g["col"] : g["col"] + SC * g["n"]],
                            func=AF.Exp,
                        ).then_inc(s_exp, 1)
                scalar.wait_ge(s_mv, NTB)
                g = GROUPS[3]
                scalar.activation(
                    out=E_G[3][:, :],
                    in_=ringA[:, g["col"] : g["col"] + SC * g["n"]],
                    func=AF.Exp,
                ).then_inc(s_exp, 1)

            @block.gpsimd
            def _(gpsimd):
                gpsimd.dma_start(
                    out=vTb[:, 2 * TK : 3 * TK], in_=vt_ext[256:384, :]
                ).then_inc(s_vt[2], 16)
                gpsimd.dma_start(out=scale_bf[:, :], in_=scl_ext[:, :]).then_inc(
                    s_scl, 16
                )
                gpsimd.dma_start(out=ident_bf[:, :], in_=idb_ext[:, :]).then_inc(
                    s_idb, 16
                )
                gpsimd.dma_start(
                    out=v_bf[:, :].rearrange("p (sc d) -> p sc d", sc=SC),
                    in_=vb_ext[:, :].rearrange("(sc p) d -> p sc d", p=128),
                ).then_inc(s_vbf, 16)

            @block.vector
            def _(vector):
                vector.memset(ones_bf[:, :], 1.0)
                # evacuations: q first, then k (the k copy separates the q_f
                # write from the adds' scalar prefetch)
                rB3 = ringB[:, :].rearrange("p (b x) -> p b x", b=4)
                vector.wait_ge(s_proj, 4)
                vector.tensor_copy(out=q_f[:, :], in_=rB3[:, 0:2, 0:128]).then_inc(
                    s_evac, 1
                )
                vector.tensor_copy(out=k_bf[:, :], in_=ringB[:, 1024:2048]).then_inc(
                    s_evac, 1
                )
                # phase 1 adds with group epilogue pieces woven in
                rc_at = {g["rc"]: i for i, g in enumerate(GROUPS) if g["rc"]}
                mul_at = {g["mul"]: i for i, g in enumerate(GROUPS) if g["mul"]}
                for tb in range(NTB):
                    buf = Xs[tb % 3]
                    if tb >= 3:
                        vector.wait_ge(s_tanh, tb - 1)
                    for tl in range(TB):
                        t = tb * TB + tl
                        for uc in range(UC):
                            ins = vector.tensor_scalar_add(
                                out=buf[
                                    :, (tl * UC + uc) * TK : (tl * UC + uc + 1) * TK
                                ],
                                in0=k_bf[:, uc * TK : (uc + 1) * TK],
                                scalar1=q_f[:, uc * 128 + t : uc * 128 + t + 1],
                            )
                        if tb == 0 and tl == 3:
                            ins.then_inc(s_add, 1)
                    ins.then_inc(s_add, 1)
                    if tb in rc_at:
                        rc_dve(vector, rc_at[tb])
                    if tb in mul_at:
                        mul_dve(vector, mul_at[tb])
                # remaining group epilogues
                rc_dve(vector, 2)
                mul_dve(vector, 2)
                rc_dve(vector, 3)
                mul_dve(vector, 3)

            @block.tensor
            def _(tensor):
                # k projection - starts as soon as vT chunks + W2 land
                tensor.wait_ge(s_w2, 16)
                for uc in range(UC):
                    for dc in range(DC):
                        if uc == 0:
                            tensor.wait_ge(s_vt[dc], 16)
                        ins = tensor.matmul(
                            out=k_ps[uc],
                            lhsT=w2b[:, dc * U + uc * 128 : dc * U + uc * 128 + 128],
                            rhs=vTb[:, dc * TK : (dc + 1) * TK],
                            start=(dc == 0),
                            stop=(dc == DC - 1),
                        )
                    ins.then_inc(s_proj, 1)
                tensor.wait_ge(s_qt, 16)
                tensor.wait_ge(s_w1, 16)
                for uc in range(UC):
                    for dc in range(DC):
                        ins = tensor.matmul(
                            out=q_ps[uc],
                            lhsT=w1b[:, dc * U + uc * 128 : dc * U + uc * 128 + 128],
                            rhs=qTb[:, dc * 128 : (dc + 1) * 128],
                            start=(dc == 0),
                            stop=(dc == DC - 1),
                        )
                    ins.then_inc(s_proj, 1)
                tensor.wait_ge(s_scl, 16)
                # phase 1: score matvecs; group phase-2 woven in
                pe_at = {g["pe"]: i for i, g in enumerate(GROUPS) if g["pe"]}
                for tb in range(NTB):
                    tensor.wait_ge(s_tanh, mv_tanh_thresh(tb, 0))
                    Tt = Ts[tb % 2]
                    for tl in range(TB):
                        if tb in (0, 15) and tl == 4:
                            tensor.wait_ge(s_tanh, mv_tanh_thresh(tb, 4))
                        t = tb * TB + tl
                        gi, g = grp_of(t)
                        col = g["col"] + (t - g["t0"])
                        for sc in range(SC):
                            for uc in range(UC):
                                base = (tl * UC + uc) * TK + sc * 128
                                ins = tensor.matmul(
                                    out=ringA[:, col + sc * g["n"] :][:, 0:1],
                                    lhsT=Tt[:, base : base + 128],
                                    rhs=scale_bf[:, uc : uc + 1],
                                    start=(uc == 0),
                                    stop=(uc == UC - 1),
                                )
                    ins.then_inc(s_mv, 1)
                    if tb in pe_at:
                        phase2_pe(tensor, pe_at[tb])
                phase2_pe(tensor, 3)

    return nc


_NC = None


def _get_nc() -> bass.Bass:
    global _NC
    if _NC is None:
        _NC = build_bass()
    return _NC


_CONST = None


def make_in_maps(query, value, W1, W2, scale):
    global _CONST
    import ml_dtypes

    bf = ml_dtypes.bfloat16
    if _CONST is None:
        _CONST = {"identb": np.eye(128).astype(bf)}
    query = np.asarray(query, dtype=np.float32)
    value = np.asarray(value, dtype=np.float32)
    W1b = np.ascontiguousarray(np.asarray(W1, np.float32).astype(bf))
    W2b = np.ascontiguousarray(np.asarray(W2, np.float32).astype(bf))
    scaleb = np.ascontiguousarray(
        np.asarray(scale, np.float32).reshape(UC, 128).T.astype(bf)
    )
    in_maps = []
    for c in range(N_CORES):
        b, th = c // 2, c % 2
        qloc = query[b, th * T_ROWS : (th + 1) * T_ROWS, :]
        vloc = value[b]
        in_maps.append(
            {
                "queryT": np.ascontiguousarray(qloc.T.astype(bf)),
                "valueT": np.ascontiguousarray(vloc.T.astype(bf)),
                "valuebf": np.ascontiguousarray(vloc.astype(bf)),
                "W1b": W1b,
                "W2b": W2b,
                "scaleb": scaleb,
                "identb": _CONST["identb"],
            }
        )
    return in_maps


def assemble(results):
    context = np.empty((B, TQ, DV), dtype=np.float32)
    attn = np.empty((B, TQ, TK), dtype=np.float32)
    for c in range(N_CORES):
        b, th = c // 2, c % 2
        context[b, th * T_ROWS : (th + 1) * T_ROWS, :] = results[c]["context"]
        attn[b, th * T_ROWS : (th + 1) * T_ROWS, :] = results[c]["attn"]
    return context, attn


def kernel(query, value, W1, W2, scale):
    nc = _get_nc()
    in_maps = make_in_maps(query, value, W1, W2, scale)
    res = run_bass_kernel_spmd(nc, in_maps, core_ids=list(range(N_CORES)))
    return assemble(res.results)
